# revision 44
# baseline (speedup 1.0000x reference)
"""Causal self-attention (B=4, S=2048, D=1024, H=16, HD=64) on 8 trn2 cores.

Sharding: core c handles batch b = c//2 and head-group g = c%2 (8 heads).
Each core computes its 8 heads' attention plus the partial output
projection over its d-slice; the host adds the two partial y's per batch.

Device layout is fully transposed ([feature, seq]) so every matmul
contraction lands on the partition dim with no on-device transposes:
  q/k   = wqk8^T @ x8         (fp8e4 DoubleRow, 2x PE rate, fp32 psum)
  v     = x^T @ wv            (bf16)
  scoresT[s_k, s_q] = k8^T @ q8   (fp8e4 DoubleRow: each head's 64-dim
                                   contraction is laid out as [32p, 2]
                                   via a host-side row permutation)
  pT = exp(scoresT/(8*256))   (ACT, bf16 out; triangular mask on boundary)
  pv[128, s_q] = v_aug^T @ pT (bf16; rows 0-63 = ones block -> replicated
                               softmax denominators, rows 64-127 = out)
  yT = wprojT^T @ (outT / denom)               (float32r)
Scales: x8 = 4x, wqk8 = 4w  ->  q8 = 16q, scores psum = 256*s; the exp
scale folds the 1/256 back out.  v/proj stay bf16/f32r so the softmax
output path keeps full precision.
QKV(n=j+1) and proj(j-1) matmul chains are interleaved into attention
column j so the PE never idles long enough for HAM to re-throttle.
"""

from contextlib import ExitStack

import ml_dtypes
import numpy as np

import concourse.bacc as bacc
import concourse.mybir as mybir
import concourse.tile as tile
from concourse._compat import with_exitstack
from concourse.bass import ds, ts  # noqa: E402
from concourse.bass_utils import run_bass_kernel_spmd
from concourse.masks import make_upper_triangular

B, S, D = 4, 2048, 1024
H, HD = 16, 64
P = 128
GH = 8            # heads per core
DS = GH * HD      # 512, d-slice per core
EQK = 2 * DS      # 1024 q+k features per core
KD = D // P       # 8 contraction subtiles for qkv
KP = DS // P      # 4 contraction subtiles for proj
NJ = S // 512     # 4 s_q tiles of 512
NST = S // P      # 16 s_k tiles of 128
F32 = mybir.dt.float32
F32R = mybir.dt.float32r
BF16 = mybir.dt.bfloat16
FP8 = mybir.dt.float8e4
EXP = mybir.ActivationFunctionType.Exp
DR = mybir.MatmulPerfMode.DoubleRow

SX = 4.0          # host scale on x8
SW = 4.0          # host scale on wqk8
# exp scale absorbing fp8 scaling; extra /2 because the score matmul feeds
# the same data through both DoubleRow k-tiles (stride-0 broadcast), which
# doubles the accumulated product.
ESCALE = 0.125 / (SX * SW) ** 2 / 2


@with_exitstack
def _emit(ctx: ExitStack, tc: tile.TileContext, xT, xT8, wqk8T, wvT, wprojT,
          yT):
    nc = tc.nc

    xT_t = xT.rearrange("(ko ki) s -> ki ko s", ki=P)      # [128, 8, 2048]
    xT8_t = xT8.rearrange("(ko ki) s -> ki ko s", ki=P)    # [128, 8, 2048]
    wqk8_t = wqk8T.rearrange("(ko ki) e -> ki ko e", ki=P)  # [128, 8, 1024]
    wv_t = wvT.rearrange("(ko ki) e -> ki ko e", ki=P)     # [128, 8, 512]
    wp_t = wprojT.rearrange("(ko ki) e -> ki ko e", ki=P)  # [128, 4, 1024]
    yT_t = yT.rearrange("(mo mi) s -> mi mo s", mi=P)      # [128, 8, 2048]

    const = ctx.enter_context(tc.tile_pool(name="const", bufs=1))
    qk_pool = ctx.enter_context(tc.tile_pool(name="qkp", bufs=1))
    big = ctx.enter_context(tc.tile_pool(name="big", bufs=1))
    pt_pool = ctx.enter_context(tc.tile_pool(name="ptp", bufs=4))
    xin = ctx.enter_context(tc.tile_pool(name="xin", bufs=2))
    x8in = ctx.enter_context(tc.tile_pool(name="x8in", bufs=8))
    ot_pool = ctx.enter_context(tc.tile_pool(name="otp", bufs=4))
    sm = ctx.enter_context(tc.tile_pool(name="sm", bufs=4))
    yout = ctx.enter_context(tc.tile_pool(name="yo", bufs=3))
    ps_sc = ctx.enter_context(tc.tile_pool(name="ps_sc", bufs=2, space="PSUM"))
    ps_pv = ctx.enter_context(tc.tile_pool(name="ps_pv", bufs=2, space="PSUM"))

    wp = const.tile([P, KP, D], F32R)
    # wqk8 split at DoubleRow k-PAIR granularity so the first QKV chain's
    # dependencies resolve per 256-KB slice instead of per whole tensor
    wqk8s = [const.tile([P, 2, EQK], FP8, name=f"wqk8_{kk}")
             for kk in range(KD // 2)]
    wv = const.tile([P, KD, DS], BF16)
    mask = const.tile([P, P], BF16)
    make_upper_triangular(nc, mask[:], val=1.0, diag=True)

    # qkT: e-tiles 0-3 = q head pairs, 4-7 = k head pairs; [e_in, tile, s]
    # fp8: the clock governor throttles on total PE duty, so halving score
    # row-work via DoubleRow keeps the whole chip's clocks up.
    qk8 = qk_pool.tile([P, 8, S], FP8)
    # v natural layout + 64-wide ones block per head: [s_in, s_tile, head, 128]
    # Ones block FIRST: PV psum rows 0-63 = denom copies, 64-127 = out.
    # (reciprocal_approx_fast drops the partition offset of its input AP, so
    # the denominators must sit at partition 0.)
    vaug = big.tile([P, NST, GH, 2 * HD], BF16)
    nc.gpsimd.memset(vaug[:, :, :, 0:HD], 1.0)

    xts = [None] * NJ
    x8ts = [None] * NJ

    def load_x(n):
        # x loads go out on the gpsimd DGE queue so they don't queue behind
        # the y stores on the sync queue
        xt = xin.tile([P, KD, 512], BF16, tag="xt", name="xt")
        nc.gpsimd.dma_start(xt[:], xT_t[:, :, ts(n, 512)])
        xts[n] = xt
        x8t = [x8in.tile([P, 2, 512], FP8, tag="x8t", name="x8t")
               for _ in range(KD // 2)]
        for kk in range(KD // 2):
            nc.gpsimd.dma_start(x8t[kk][:],
                                xT8_t[:, 2 * kk:2 * kk + 2, ts(n, 512)])
        x8ts[n] = x8t

    def qkv_qk_chain(n, m):
        ps = ps_sc.tile([P, 1024], F32, tag="sc", name="ps")[:, 0:512]
        for kk in range(KD // 2):
            nc.tensor.matmul(ps[:], wqk8s[kk][:, :, ts(m, P)],
                             x8ts[n][kk][:],
                             start=(kk == 0), stop=(kk == KD // 2 - 1),
                             perf_mode=DR)
        nc.vector.tensor_copy(qk8[:, m, ts(n, 512)], ps[:])

    def qkv_v_chain(n, ss):
        st = n * 4 + ss
        ps = ps_sc.tile([P, 1024], F32, tag="sc", name="ps")[:, 0:512]
        for k in range(KD):
            nc.tensor.matmul(ps[:], xts[n][:, k, ts(ss, P)], wv[:, k, :],
                             start=(k == 0), stop=(k == KD - 1))
        nc.vector.tensor_copy(vaug[:, st, :, HD:],
                              ps.rearrange("p (h d) -> p h d", h=GH))

    outTs = [None] * NJ

    def _emit_scores(l, j, i):
        """Score matmuls + exp + boundary mask for (pair l, column j, tile i).
        Returns (pt, off) for the matching PV step."""
        t = i - 4 * j  # >=0 -> diagonal boundary tile
        off = 128 * t if t > 0 else 0
        sc = ps_sc.tile([P, 1024], F32, tag="sc", name="sc")
        scv = sc.rearrange("p (u f) -> p u f", u=2)
        # fp8 DoubleRow at K=64: both DoubleRow k-tiles read the SAME
        # data via a stride-0 broadcast (out = 2*k.q, folded into ESCALE)
        nc.tensor.matmul(
            sc[:, off:512],
            qk8[0:64, 4 + l:5 + l, ts(i, P)].to_broadcast((64, 2, P)),
            qk8[0:64, l:l + 1,
                ds(j * 512 + off, 512 - off)].to_broadcast(
                    (64, 2, 512 - off)),
            start=True, stop=True, perf_mode=DR)
        nc.tensor.matmul(
            sc[:, 512 + off:1024],
            qk8[64:128, 4 + l:5 + l, ts(i, P)].to_broadcast((64, 2, P)),
            qk8[64:128, l:l + 1,
                ds(j * 512 + off, 512 - off)].to_broadcast(
                    (64, 2, 512 - off)),
            start=True, stop=True, perf_mode=DR)
        pt = pt_pool.tile([P, 1024], BF16, tag="pt", name="pt")
        ptv = pt.rearrange("p (u f) -> p u f", u=2)
        nc.scalar.activation(ptv[:, :, off:512], scv[:, :, off:512],
                             EXP, scale=ESCALE)
        if t >= 0:  # causal mask on the boundary 128-col block
            nc.vector.tensor_tensor(
                ptv[:, :, off:off + P], ptv[:, :, off:off + P],
                mask[:, None, :].to_broadcast((P, 2, P)),
                mybir.AluOpType.mult)
        return pt, off

    def _normalize(l, j, pv):
        outT = outTs[j]
        for hh in (0, 1):
            half = pv[:, 512 * hh:512 * (hh + 1)]
            rec = sm.tile([HD, 512], F32, tag="rec", name="rec")
            nc.vector.reciprocal_approx_fast(rec[:], half[0:HD, :])
            nc.vector.tensor_tensor(outT[hh * HD:(hh + 1) * HD, l, :],
                                    half[HD:2 * HD, :], rec[:],
                                    mybir.AluOpType.mult)

    def proj_col_chain(j, m):
        ps = ps_sc.tile([P, 1024], F32, tag="sc", name="ps")[:, 0:512]
        for k in range(KP):
            nc.tensor.matmul(ps[:], wp[:, k, ts(m, P)], outTs[j][:, k, :],
                             start=(k == 0), stop=(k == KP - 1))
        yt = yout.tile([P, 512], BF16, tag="yt", name="yt")
        nc.vector.tensor_copy(yt[:], ps[:])
        # alternate DGE queues so the final column's stores drain in parallel
        eng = nc.sync if m % 2 == 0 else nc.gpsimd
        eng.dma_start(yT_t[:, m, ts(j, 512)], yt[:])

    def proj(j):
        for m in range(8):
            proj_col_chain(j, m)

    class Pacer:
        # Bresenham-paced emission of filler matmul chains between
        # attention iterations, to keep the PE dense (HAM stays warm).
        # Urgent thunks (deferred softmax normalizes) fire one per tick
        # ahead of the paced stream so DVE recips interleave with, not
        # ahead of, the next pair's mask multiplies.
        def __init__(self, thunks, total_ticks):
            self.thunks = list(thunks)
            self.total = max(1, total_ticks)
            self.ticks = 0
            self.fired = 0
            self.urgent = []

        def inject(self, thunks):
            self.urgent.extend(thunks)

        def tick(self):
            self.ticks += 1
            if self.urgent:
                self.urgent.pop(0)()
                return
            while (self.fired < len(self.thunks)
                   and self.fired * self.total < self.ticks * len(self.thunks)):
                self.thunks[self.fired]()
                self.fired += 1

        def flush(self):
            for t in self.urgent:
                t()
            self.urgent = []
            while self.fired < len(self.thunks):
                self.thunks[self.fired]()
                self.fired += 1

    # prelude DMAs: wqk8 k-pairs + x8 k-pairs interleaved on the sync queue
    # (the qk chains' critical path); wv + bf16 x on the gpsimd DGE queue in
    # parallel; wp on the vector queue (needed last, at proj time).
    xt0 = xin.tile([P, KD, 512], BF16, tag="xt", name="xt")
    x8t0 = [x8in.tile([P, 2, 512], FP8, tag="x8t", name="x8t")
            for _ in range(KD // 2)]
    for kk in range(KD // 2):
        nc.sync.dma_start(wqk8s[kk][:], wqk8_t[:, 2 * kk:2 * kk + 2, :])
        nc.sync.dma_start(x8t0[kk][:], xT8_t[:, 2 * kk:2 * kk + 2, ts(0, 512)])
    for k in range(KD):
        nc.gpsimd.dma_start(wv[:, k, :], wv_t[:, k, :])
        nc.gpsimd.dma_start(xt0[:, k, :], xT_t[:, k, ts(0, 512)])
    xts[0] = xt0
    x8ts[0] = x8t0
    nc.scalar.dma_start(wp[:], wp_t)

    # prelude: QKV for the first s-block
    for m in range(8):
        qkv_qk_chain(0, m)
    for ss in range(4):
        qkv_v_chain(0, ss)

    # Flattened, software-pipelined attention stream across ALL columns.
    # Filler allocation: qkv(j+1) is pinned to column j (needed at column
    # j+1); the proj chains are all deferred to the LAST column, which has
    # 40% of the attention iterations (and hence PE slack) but no qkv work.
    for j in range(NJ):
        outTs[j] = ot_pool.tile([P, KP, 512], F32R, tag="outT", name="outT")
    pacers = []
    for j in range(NJ):
        thunks = []
        if j + 1 < NJ:
            for m in range(8):
                thunks.append(lambda n=j + 1, m=m: qkv_qk_chain(n, m))
            for ss in range(4):
                thunks.append(lambda n=j + 1, ss=ss: qkv_v_chain(n, ss))
        else:
            for jj in range(NJ - 1):
                for m in range(8):
                    thunks.append(lambda jj=jj, m=m: proj_col_chain(jj, m))
        pacers.append(Pacer(thunks, 4 * 4 * (j + 1)))

    all_items = [(j, l, i) for j in range(NJ) for l in range(4)
                 for i in range(4 * (j + 1))]
    load_x(1)
    pvs = {}
    cur = _emit_scores(all_items[0][1], all_items[0][0], all_items[0][2])
    for idx, (j, l, i) in enumerate(all_items):
        imax = 4 * (j + 1)
        nxt = None
        if idx + 1 < len(all_items):
            jn, ln, i_n = all_items[idx + 1]
            if jn != j:
                # column boundary: the next column's scores read qk8 written
                # by this column's qkv chains -- flush them FIRST so the
                # in-order PE queue never waits on work queued behind it.
                pacers[j].flush()
                if jn + 1 < NJ:
                    load_x(jn + 1)
            nxt = _emit_scores(ln, jn, i_n)
        pacers[j].tick()
        pt, off = cur
        if i == 0:
            pvs[(j, l)] = ps_pv.tile([P, 1024], F32, tag="pv", name="pv")
        pv = pvs[(j, l)]
        nc.tensor.matmul(pv[:, off:512], vaug[:, i, 2 * l, :],
                         pt[:, off:512],
                         start=(i == 0), stop=(i == imax - 1))
        nc.tensor.matmul(pv[:, 512 + off:1024], vaug[:, i, 2 * l + 1, :],
                         pt[:, 512 + off:1024],
                         start=(i == 0), stop=(i == imax - 1))
        if i == imax - 1:
            _normalize(l, j, pvs.pop((j, l)))
        cur = nxt
    pacers[NJ - 1].flush()
    proj(NJ - 1)


_NC = None


def build_nc():
    global _NC
    if _NC is not None:
        return _NC
    nc = bacc.Bacc("TRN2", target_bir_lowering=False, debug=False)
    xT = nc.dram_tensor("xT", [D, S], BF16, kind="ExternalInput")
    xT8 = nc.dram_tensor("xT8", [D, S], FP8, kind="ExternalInput")
    wqk8T = nc.dram_tensor("wqk8T", [D, EQK], FP8, kind="ExternalInput")
    wvT = nc.dram_tensor("wvT", [D, DS], BF16, kind="ExternalInput")
    wprojT = nc.dram_tensor("wprojT", [DS, D], F32R, kind="ExternalInput")
    yT = nc.dram_tensor("yT", [D, S], BF16, kind="ExternalOutput")
    with tile.TileContext(nc) as tc:
        _emit(tc, xT.ap(), xT8.ap(), wqk8T.ap(), wvT.ap(), wprojT.ap(),
              yT.ap())
    nc.compile()
    _NC = nc
    return nc


def make_in_maps(x, w_attn, w_proj):
    x = np.ascontiguousarray(np.asarray(x, dtype=np.float32))
    w_attn = np.asarray(w_attn, dtype=np.float32)
    w_proj = np.asarray(w_proj, dtype=np.float32)
    in_maps = []
    for c in range(8):
        b, g = divmod(c, 2)
        rows = slice(g * DS, (g + 1) * DS)
        wq_c = w_attn[0 * D:1 * D][rows] * SW           # [512, 1024]
        wk_c = w_attn[1 * D:2 * D][rows] * SW
        wqk8_c = np.concatenate([wq_c, wk_c], axis=0)   # [1024, 1024]
        wv_c = w_attn[2 * D:3 * D][rows]                # [512, 1024]
        xTb = np.ascontiguousarray(x[b].T)
        in_maps.append({
            "xT": xTb.astype(ml_dtypes.bfloat16),
            "xT8": (xTb * SX).astype(ml_dtypes.float8_e4m3),
            "wqk8T": np.ascontiguousarray(wqk8_c.T).astype(
                ml_dtypes.float8_e4m3),
            "wvT": np.ascontiguousarray(wv_c.T).astype(ml_dtypes.bfloat16),
            "wprojT": np.ascontiguousarray(w_proj[:, rows].T),  # [512, 1024]
        })
    return in_maps


def gather(results):
    y = np.empty((B, S, D), dtype=np.float32)
    for b in range(B):
        yT = (results[2 * b]["yT"].astype(np.float32)
              + results[2 * b + 1]["yT"].astype(np.float32))
        y[b] = yT.T
    return y


def run(x, w_attn, w_proj, trace=False, tmpdir=None):
    nc = build_nc()
    in_maps = make_in_maps(x, w_attn, w_proj)
    res = run_bass_kernel_spmd(nc, in_maps, list(range(8)),
                               trace=trace, tmpdir=tmpdir)
    return gather(res.results), res


def kernel(x, w_attn, w_proj):
    y, _ = run(x, w_attn, w_proj)
    return y


# revision 46
# speedup vs baseline: 1.1780x; 1.1780x over previous
"""Causal self-attention (B=4, S=2048, D=1024, H=16, HD=64) on 8 trn2 cores.

Sharding: core c handles batch b = c//2 and head-group g = c%2 (8 heads).
Each core computes its 8 heads' attention plus the partial output
projection over its d-slice; the host adds the two partial y's per batch.

Device layout is fully transposed ([feature, seq]) so every matmul
contraction lands on the partition dim with no on-device transposes:
  q/k   = wqk8^T @ x8         (fp8e4 DoubleRow, 2x PE rate, fp32 psum)
  v     = x^T @ wv            (bf16)
  scoresT[s_k, s_q] = k8^T @ q8   (fp8e4 DoubleRow: each head's 64-dim
                                   contraction is laid out as [32p, 2]
                                   via a host-side row permutation)
  pT = exp(scoresT/(8*256))   (ACT, bf16 out; triangular mask on boundary)
  pv[128, s_q] = v_aug^T @ pT (bf16; rows 0-63 = ones block -> replicated
                               softmax denominators, rows 64-127 = out)
  yT = wprojT^T @ (outT / denom)               (float32r)
Scales: x8 = 4x, wqk8 = 4w  ->  q8 = 16q, scores psum = 256*s; the exp
scale folds the 1/256 back out.  v/proj stay bf16/f32r so the softmax
output path keeps full precision.
QKV(n=j+1) and proj(j-1) matmul chains are interleaved into attention
column j so the PE never idles long enough for HAM to re-throttle.
"""

from contextlib import ExitStack

import ml_dtypes
import numpy as np

import concourse.bacc as bacc
import concourse.mybir as mybir
import concourse.tile as tile
from concourse._compat import with_exitstack
from concourse.bass import ds, ts  # noqa: E402
from concourse.bass_utils import run_bass_kernel_spmd
from concourse.masks import make_upper_triangular

B, S, D = 4, 2048, 1024
H, HD = 16, 64
P = 128
GH = 8            # heads per core
DS = GH * HD      # 512, d-slice per core
EQK = 2 * DS      # 1024 q+k features per core
KD = D // P       # 8 contraction subtiles for qkv
KP = DS // P      # 4 contraction subtiles for proj
NJ = S // 512     # 4 s_q tiles of 512
NST = S // P      # 16 s_k tiles of 128
F32 = mybir.dt.float32
F32R = mybir.dt.float32r
BF16 = mybir.dt.bfloat16
FP8 = mybir.dt.float8e4
EXP = mybir.ActivationFunctionType.Exp
DR = mybir.MatmulPerfMode.DoubleRow

SX = 4.0          # host scale on x8
SW = 4.0          # host scale on wqk8
# exp scale absorbing fp8 scaling; extra /2 because the score matmul feeds
# the same data through both DoubleRow k-tiles (stride-0 broadcast), which
# doubles the accumulated product.
ESCALE = 0.125 / (SX * SW) ** 2 / 2


@with_exitstack
def _emit(ctx: ExitStack, tc: tile.TileContext, xT, xT8, wqk8T, wvT, wprojT,
          yT):
    nc = tc.nc

    xT_t = xT.rearrange("(ko ki) s -> ki ko s", ki=P)      # [128, 8, 2048]
    xT8_t = xT8.rearrange("(ko ki) s -> ki ko s", ki=P)    # [128, 8, 2048]
    wqk8_t = wqk8T.rearrange("(ko ki) e -> ki ko e", ki=P)  # [128, 8, 1024]
    wv_t = wvT.rearrange("(ko ki) e -> ki ko e", ki=P)     # [128, 8, 512]
    wp_t = wprojT.rearrange("(ko ki) e -> ki ko e", ki=P)  # [128, 4, 1024]
    yT_t = yT.rearrange("(mo mi) s -> mi mo s", mi=P)      # [128, 8, 2048]

    const = ctx.enter_context(tc.tile_pool(name="const", bufs=1))
    qk_pool = ctx.enter_context(tc.tile_pool(name="qkp", bufs=1))
    big = ctx.enter_context(tc.tile_pool(name="big", bufs=1))
    pt_pool = ctx.enter_context(tc.tile_pool(name="ptp", bufs=4))
    xin = ctx.enter_context(tc.tile_pool(name="xin", bufs=2))
    x8in = ctx.enter_context(tc.tile_pool(name="x8in", bufs=8))
    ot_pool = ctx.enter_context(tc.tile_pool(name="otp", bufs=4))
    sm = ctx.enter_context(tc.tile_pool(name="sm", bufs=4))
    yout = ctx.enter_context(tc.tile_pool(name="yo", bufs=3))
    ps_sc = ctx.enter_context(tc.tile_pool(name="ps_sc", bufs=2, space="PSUM"))
    ps_pv = ctx.enter_context(tc.tile_pool(name="ps_pv", bufs=2, space="PSUM"))

    wp = const.tile([P, KP, D], F32R)
    # wqk8 split at DoubleRow k-PAIR granularity so the first QKV chain's
    # dependencies resolve per 256-KB slice instead of per whole tensor
    wqk8s = [const.tile([P, 2, EQK], FP8, name=f"wqk8_{kk}")
             for kk in range(KD // 2)]
    wv = const.tile([P, KD, DS], BF16)
    mask = const.tile([P, P], BF16)
    make_upper_triangular(nc, mask[:], val=1.0, diag=True)

    # qkT: e-tiles 0-3 = q head pairs, 4-7 = k head pairs; [e_in, tile, s]
    # fp8: the clock governor throttles on total PE duty, so halving score
    # row-work via DoubleRow keeps the whole chip's clocks up.
    qk8 = qk_pool.tile([P, 8, S], FP8)
    # v natural layout + 64-wide ones block per head: [s_in, s_tile, head, 128]
    # Ones block FIRST: PV psum rows 0-63 = denom copies, 64-127 = out.
    # (reciprocal_approx_fast drops the partition offset of its input AP, so
    # the denominators must sit at partition 0.)
    vaug = big.tile([P, NST, GH, 2 * HD], BF16)
    nc.gpsimd.memset(vaug[:, :, :, 0:HD], 1.0)

    xts = [None] * NJ
    x8ts = [None] * NJ

    def load_x(n):
        # x loads go out on the gpsimd DGE queue so they don't queue behind
        # the y stores on the sync queue
        xt = xin.tile([P, KD, 512], BF16, tag="xt", name="xt")
        nc.gpsimd.dma_start(xt[:], xT_t[:, :, ts(n, 512)])
        xts[n] = xt
        x8t = [x8in.tile([P, 2, 512], FP8, tag="x8t", name="x8t")
               for _ in range(KD // 2)]
        for kk in range(KD // 2):
            nc.gpsimd.dma_start(x8t[kk][:],
                                xT8_t[:, 2 * kk:2 * kk + 2, ts(n, 512)])
        x8ts[n] = x8t

    def qkv_qk_chain(n, m):
        ps = ps_sc.tile([P, 1024], F32, tag="sc", name="ps")[:, 0:512]
        for kk in range(KD // 2):
            nc.tensor.matmul(ps[:], wqk8s[kk][:, :, ts(m, P)],
                             x8ts[n][kk][:],
                             start=(kk == 0), stop=(kk == KD // 2 - 1),
                             perf_mode=DR)
        nc.vector.tensor_copy(qk8[:, m, ts(n, 512)], ps[:])

    def qkv_v_chain(n, ss):
        st = n * 4 + ss
        ps = ps_sc.tile([P, 1024], F32, tag="sc", name="ps")[:, 0:512]
        for k in range(KD):
            nc.tensor.matmul(ps[:], xts[n][:, k, ts(ss, P)], wv[:, k, :],
                             start=(k == 0), stop=(k == KD - 1))
        nc.vector.tensor_copy(vaug[:, st, :, HD:],
                              ps.rearrange("p (h d) -> p h d", h=GH))

    outTs = [None] * NJ

    def _emit_scores(l, j, i):
        """Score matmuls + exp + boundary mask for (pair l, column j, tile i).
        Returns (pt, off) for the matching PV step."""
        t = i - 4 * j  # >=0 -> diagonal boundary tile
        off = 128 * t if t > 0 else 0
        sc = ps_sc.tile([P, 1024], F32, tag="sc", name="sc")
        scv = sc.rearrange("p (u f) -> p u f", u=2)
        # fp8 DoubleRow at K=64: both DoubleRow k-tiles read the SAME
        # data via a stride-0 broadcast (out = 2*k.q, folded into ESCALE)
        nc.tensor.matmul(
            sc[:, off:512],
            qk8[0:64, 4 + l:5 + l, ts(i, P)].to_broadcast((64, 2, P)),
            qk8[0:64, l:l + 1,
                ds(j * 512 + off, 512 - off)].to_broadcast(
                    (64, 2, 512 - off)),
            start=True, stop=True, perf_mode=DR)
        nc.tensor.matmul(
            sc[:, 512 + off:1024],
            qk8[64:128, 4 + l:5 + l, ts(i, P)].to_broadcast((64, 2, P)),
            qk8[64:128, l:l + 1,
                ds(j * 512 + off, 512 - off)].to_broadcast(
                    (64, 2, 512 - off)),
            start=True, stop=True, perf_mode=DR)
        pt = pt_pool.tile([P, 1024], BF16, tag="pt", name="pt")
        ptv = pt.rearrange("p (u f) -> p u f", u=2)
        nc.scalar.activation(ptv[:, :, off:512], scv[:, :, off:512],
                             EXP, scale=ESCALE)
        if t >= 0:  # causal mask on the boundary 128-col block
            nc.vector.tensor_tensor(
                ptv[:, :, off:off + P], ptv[:, :, off:off + P],
                mask[:, None, :].to_broadcast((P, 2, P)),
                mybir.AluOpType.mult)
        return pt, off

    def _normalize(l, j, pv):
        outT = outTs[j]
        for hh in (0, 1):
            half = pv[:, 512 * hh:512 * (hh + 1)]
            rec = sm.tile([HD, 512], F32, tag="rec", name="rec")
            nc.vector.reciprocal_approx_fast(rec[:], half[0:HD, :])
            nc.vector.tensor_tensor(outT[hh * HD:(hh + 1) * HD, l, :],
                                    half[HD:2 * HD, :], rec[:],
                                    mybir.AluOpType.mult)

    def proj_col_chain(j, m):
        ps = ps_sc.tile([P, 1024], F32, tag="sc", name="ps")[:, 0:512]
        for k in range(KP):
            nc.tensor.matmul(ps[:], wp[:, k, ts(m, P)], outTs[j][:, k, :],
                             start=(k == 0), stop=(k == KP - 1))
        yt = yout.tile([P, 512], BF16, tag="yt", name="yt")
        nc.vector.tensor_copy(yt[:], ps[:])
        # alternate DGE queues so the final column's stores drain in parallel
        eng = nc.sync if m % 2 == 0 else nc.gpsimd
        eng.dma_start(yT_t[:, m, ts(j, 512)], yt[:])

    def proj(j):
        for m in range(8):
            proj_col_chain(j, m)

    class Pacer:
        # Bresenham-paced emission of filler matmul chains between
        # attention iterations, to keep the PE dense (HAM stays warm).
        # Urgent thunks (deferred softmax normalizes) fire one per tick
        # ahead of the paced stream so DVE recips interleave with, not
        # ahead of, the next pair's mask multiplies.
        def __init__(self, thunks, total_ticks):
            self.thunks = list(thunks)
            self.total = max(1, total_ticks)
            self.ticks = 0
            self.fired = 0
            self.urgent = []

        def inject(self, thunks):
            self.urgent.extend(thunks)

        def tick(self):
            self.ticks += 1
            if self.urgent:
                self.urgent.pop(0)()
                return
            while (self.fired < len(self.thunks)
                   and self.fired * self.total < self.ticks * len(self.thunks)):
                self.thunks[self.fired]()
                self.fired += 1

        def flush(self):
            for t in self.urgent:
                t()
            self.urgent = []
            while self.fired < len(self.thunks):
                self.thunks[self.fired]()
                self.fired += 1

    # prelude DMAs: wqk8 k-pairs + x8 k-pairs interleaved on the sync queue
    # (the qk chains' critical path); wv + bf16 x on the gpsimd DGE queue in
    # parallel; wp on the vector queue (needed last, at proj time).
    xt0 = xin.tile([P, KD, 512], BF16, tag="xt", name="xt")
    x8t0 = [x8in.tile([P, 2, 512], FP8, tag="x8t", name="x8t")
            for _ in range(KD // 2)]
    for kk in range(KD // 2):
        nc.sync.dma_start(wqk8s[kk][:], wqk8_t[:, 2 * kk:2 * kk + 2, :])
        nc.sync.dma_start(x8t0[kk][:], xT8_t[:, 2 * kk:2 * kk + 2, ts(0, 512)])
    for k in range(KD):
        nc.gpsimd.dma_start(wv[:, k, :], wv_t[:, k, :])
        nc.gpsimd.dma_start(xt0[:, k, :], xT_t[:, k, ts(0, 512)])
    xts[0] = xt0
    x8ts[0] = x8t0
    nc.scalar.dma_start(wp[:], wp_t)

    # prelude: QKV for the first s-block
    for m in range(8):
        qkv_qk_chain(0, m)
    for ss in range(4):
        qkv_v_chain(0, ss)

    # Flattened, software-pipelined attention stream across ALL columns.
    # Filler allocation: qkv(j+1) is pinned to column j (needed at column
    # j+1); the proj chains are all deferred to the LAST column, which has
    # 40% of the attention iterations (and hence PE slack) but no qkv work.
    for j in range(NJ):
        outTs[j] = ot_pool.tile([P, KP, 512], F32R, tag="outT", name="outT")
    pacers = []
    for j in range(NJ):
        thunks = []
        if j + 1 < NJ:
            for m in range(8):
                thunks.append(lambda n=j + 1, m=m: qkv_qk_chain(n, m))
            for ss in range(4):
                thunks.append(lambda n=j + 1, ss=ss: qkv_v_chain(n, ss))
        else:
            for jj in range(NJ - 1):
                for m in range(8):
                    thunks.append(lambda jj=jj, m=m: proj_col_chain(jj, m))
        pacers.append(Pacer(thunks, 4 * 4 * (j + 1)))

    all_items = [(j, l, i) for j in range(NJ) for l in range(4)
                 for i in range(4 * (j + 1))]
    load_x(1)
    pvs = {}
    cur = _emit_scores(all_items[0][1], all_items[0][0], all_items[0][2])
    for idx, (j, l, i) in enumerate(all_items):
        imax = 4 * (j + 1)
        nxt = None
        if idx + 1 < len(all_items):
            jn, ln, i_n = all_items[idx + 1]
            if jn != j:
                # column boundary: the next column's scores read qk8 written
                # by this column's qkv chains -- flush them FIRST so the
                # in-order PE queue never waits on work queued behind it.
                pacers[j].flush()
                if jn + 1 < NJ:
                    load_x(jn + 1)
            nxt = _emit_scores(ln, jn, i_n)
        pacers[j].tick()
        pt, off = cur
        if i == 0:
            pvs[(j, l)] = ps_pv.tile([P, 1024], F32, tag="pv", name="pv")
        pv = pvs[(j, l)]
        nc.tensor.matmul(pv[:, off:512], vaug[:, i, 2 * l, :],
                         pt[:, off:512],
                         start=(i == 0), stop=(i == imax - 1))
        nc.tensor.matmul(pv[:, 512 + off:1024], vaug[:, i, 2 * l + 1, :],
                         pt[:, 512 + off:1024],
                         start=(i == 0), stop=(i == imax - 1))
        if i == imax - 1:
            _normalize(l, j, pvs.pop((j, l)))
        cur = nxt
    pacers[NJ - 1].flush()
    proj(NJ - 1)


_NC = None


def build_nc():
    global _NC
    if _NC is not None:
        return _NC
    nc = bacc.Bacc("TRN2", target_bir_lowering=False, debug=False)
    xT = nc.dram_tensor("xT", [D, S], BF16, kind="ExternalInput")
    xT8 = nc.dram_tensor("xT8", [D, S], FP8, kind="ExternalInput")
    wqk8T = nc.dram_tensor("wqk8T", [D, EQK], FP8, kind="ExternalInput")
    wvT = nc.dram_tensor("wvT", [D, DS], BF16, kind="ExternalInput")
    wprojT = nc.dram_tensor("wprojT", [DS, D], F32R, kind="ExternalInput")
    yT = nc.dram_tensor("yT", [D, S], BF16, kind="ExternalOutput")
    with tile.TileContext(nc) as tc:
        _emit(tc, xT.ap(), xT8.ap(), wqk8T.ap(), wvT.ap(), wprojT.ap(),
              yT.ap())
    nc.compile()
    _NC = nc
    return nc


def make_in_maps(x, w_attn, w_proj):
    x = np.ascontiguousarray(np.asarray(x, dtype=np.float32))
    w_attn = np.asarray(w_attn, dtype=np.float32)
    w_proj = np.asarray(w_proj, dtype=np.float32)
    in_maps = []
    for c in range(8):
        b, g = divmod(c, 2)
        rows = slice(g * DS, (g + 1) * DS)
        wq_c = w_attn[0 * D:1 * D][rows] * SW           # [512, 1024]
        wk_c = w_attn[1 * D:2 * D][rows] * SW
        wqk8_c = np.concatenate([wq_c, wk_c], axis=0)   # [1024, 1024]
        wv_c = w_attn[2 * D:3 * D][rows]                # [512, 1024]
        xTb = np.ascontiguousarray(x[b].T)
        in_maps.append({
            "xT": xTb.astype(ml_dtypes.bfloat16),
            "xT8": (xTb * SX).astype(ml_dtypes.float8_e4m3),
            "wqk8T": np.ascontiguousarray(wqk8_c.T).astype(
                ml_dtypes.float8_e4m3),
            "wvT": np.ascontiguousarray(wv_c.T).astype(ml_dtypes.bfloat16),
            "wprojT": np.ascontiguousarray(w_proj[:, rows].T),  # [512, 1024]
        })
    return in_maps


def gather(results):
    y = np.empty((B, S, D), dtype=np.float32)
    for b in range(B):
        yT = (results[2 * b]["yT"].astype(np.float32)
              + results[2 * b + 1]["yT"].astype(np.float32))
        y[b] = yT.T
    return y


def run(x, w_attn, w_proj, trace=False, tmpdir=None):
    nc = build_nc()
    in_maps = make_in_maps(x, w_attn, w_proj)
    res = run_bass_kernel_spmd(nc, in_maps, list(range(8)),
                               trace=trace, tmpdir=tmpdir)
    return gather(res.results), res


def kernel(x, w_attn, w_proj):
    y, _ = run(x, w_attn, w_proj)
    return y


# revision 51
# speedup vs baseline: 1.2135x; 1.0301x over previous
"""Causal self-attention (B=4, S=2048, D=1024, H=16, HD=64) on 8 trn2 cores.

Sharding: core c handles batch b = c//2 and head-group g = c%2 (8 heads).
Each core computes its 8 heads' attention plus the partial output
projection over its d-slice; the host adds the two partial y's per batch.

Device layout is fully transposed ([feature, seq]) so every matmul
contraction lands on the partition dim with no on-device transposes:
  q/k   = wqk8^T @ x8         (fp8e4 DoubleRow, 2x PE rate, fp32 psum)
  v     = x^T @ wv            (bf16)
  scoresT[s_k, s_q] = k8^T @ q8   (fp8e4 DoubleRow at K=64: both DR
                                   k-tiles read the same data via a
                                   stride-0 broadcast; the 2x product
                                   is folded into ESCALE)
  pT = exp(scoresT/(8*256))   (ACT, bf16 out; triangular mask on boundary)
  pv[128, s_q] = v_aug^T @ pT (bf16; rows 0-63 = ones block -> replicated
                               softmax denominators, rows 64-127 = out)
  yT = wprojT^T @ (outT / denom)               (float32r)
Scales: x8 = 4x, wqk8 = 4w  ->  q8 = 16q, scores psum = 256*s; the exp
scale folds the 1/256 back out.  v/proj stay bf16/f32r so the softmax
output path keeps full precision.
QKV(n=j+1) and proj(j-1) matmul chains are interleaved into attention
column j so the PE never idles long enough for HAM to re-throttle.
"""

from collections import deque
from contextlib import ExitStack

import ml_dtypes
import numpy as np

import concourse.bacc as bacc
import concourse.mybir as mybir
import concourse.tile as tile
from concourse._compat import with_exitstack
from concourse.bass import ds, ts  # noqa: E402
from concourse.bass_utils import run_bass_kernel_spmd
from concourse.masks import make_upper_triangular

B, S, D = 4, 2048, 1024
H, HD = 16, 64
P = 128
GH = 8            # heads per core
DS = GH * HD      # 512, d-slice per core
EQK = 2 * DS      # 1024 q+k features per core
KD = D // P       # 8 contraction subtiles for qkv
KP = DS // P      # 4 contraction subtiles for proj
NJ = S // 512     # 4 s_q tiles of 512
NST = S // P      # 16 s_k tiles of 128
F32 = mybir.dt.float32
F32R = mybir.dt.float32r
BF16 = mybir.dt.bfloat16
FP8 = mybir.dt.float8e4
EXP = mybir.ActivationFunctionType.Exp
DR = mybir.MatmulPerfMode.DoubleRow

SX = 4.0          # host scale on x8
SW = 4.0          # host scale on wqk8
# exp scale absorbing fp8 scaling; extra /2 because the score matmul feeds
# the same data through both DoubleRow k-tiles (stride-0 broadcast), which
# doubles the accumulated product.
ESCALE = 0.125 / (SX * SW) ** 2 / 2


@with_exitstack
def _emit(ctx: ExitStack, tc: tile.TileContext, xT, xT8, wqk8T, wvT, wprojT,
          yT):
    nc = tc.nc

    xT_t = xT.rearrange("(ko ki) s -> ki ko s", ki=P)      # [128, 8, 2048]
    xT8_t = xT8.rearrange("(ko ki) s -> ki ko s", ki=P)    # [128, 8, 2048]
    wqk8_t = wqk8T.rearrange("(ko ki) e -> ki ko e", ki=P)  # [128, 8, 1024]
    wv_t = wvT.rearrange("(ko ki) e -> ki ko e", ki=P)     # [128, 8, 512]
    wp_t = wprojT.rearrange("(ko ki) e -> ki ko e", ki=P)  # [128, 4, 1024]
    yT_t = yT.rearrange("(mo mi) s -> mi mo s", mi=P)      # [128, 8, 2048]

    const = ctx.enter_context(tc.tile_pool(name="const", bufs=1))
    qk_pool = ctx.enter_context(tc.tile_pool(name="qkp", bufs=1))
    big = ctx.enter_context(tc.tile_pool(name="big", bufs=1))
    pt_pool = ctx.enter_context(tc.tile_pool(name="ptp", bufs=8))
    xin = ctx.enter_context(tc.tile_pool(name="xin", bufs=2))
    x8in = ctx.enter_context(tc.tile_pool(name="x8in", bufs=8))
    ot_pool = ctx.enter_context(tc.tile_pool(name="otp", bufs=4))
    sm = ctx.enter_context(tc.tile_pool(name="sm", bufs=4))
    yout = ctx.enter_context(tc.tile_pool(name="yo", bufs=3))
    ps_sc = ctx.enter_context(tc.tile_pool(name="ps_sc", bufs=2, space="PSUM"))
    ps_pv = ctx.enter_context(tc.tile_pool(name="ps_pv", bufs=2, space="PSUM"))

    wp = const.tile([P, KP, D], F32R)
    # wqk8 split at DoubleRow k-PAIR granularity so the first QKV chain's
    # dependencies resolve per 256-KB slice instead of per whole tensor
    wqk8s = [const.tile([P, 2, EQK], FP8, name=f"wqk8_{kk}")
             for kk in range(KD // 2)]
    wv = const.tile([P, KD, DS], BF16)
    mask = const.tile([P, P], BF16)
    make_upper_triangular(nc, mask[:], val=1.0, diag=True)

    # qkT: e-tiles 0-3 = q head pairs, 4-7 = k head pairs; [e_in, tile, s]
    # fp8: the clock governor throttles on total PE duty, so halving score
    # row-work via DoubleRow keeps the whole chip's clocks up.
    qk8 = qk_pool.tile([P, 8, S], FP8)
    # v natural layout + 64-wide ones block per head: [s_in, s_tile, head, 128]
    # Ones block FIRST: PV psum rows 0-63 = denom copies, 64-127 = out.
    # (reciprocal_approx_fast drops the partition offset of its input AP, so
    # the denominators must sit at partition 0.)
    vaug = big.tile([P, NST, GH, 2 * HD], BF16)
    nc.gpsimd.memset(vaug[:, :, :, 0:HD], 1.0)

    xts = [None] * NJ
    x8ts = [None] * NJ

    def load_x(n):
        # x loads go out on the gpsimd DGE queue so they don't queue behind
        # the y stores on the sync queue
        xt = xin.tile([P, KD, 512], BF16, tag="xt", name="xt")
        nc.gpsimd.dma_start(xt[:], xT_t[:, :, ts(n, 512)])
        xts[n] = xt
        x8t = [x8in.tile([P, 2, 512], FP8, tag="x8t", name="x8t")
               for _ in range(KD // 2)]
        for kk in range(KD // 2):
            nc.gpsimd.dma_start(x8t[kk][:],
                                xT8_t[:, 2 * kk:2 * kk + 2, ts(n, 512)])
        x8ts[n] = x8t

    def qkv_qk_chain(n, m):
        ps = ps_sc.tile([P, 1024], F32, tag="sc", name="ps")[:, 0:512]
        for kk in range(KD // 2):
            nc.tensor.matmul(ps[:], wqk8s[kk][:, :, ts(m, P)],
                             x8ts[n][kk][:],
                             start=(kk == 0), stop=(kk == KD // 2 - 1),
                             perf_mode=DR)
        nc.vector.tensor_copy(qk8[:, m, ts(n, 512)], ps[:])

    def qkv_v_chain(n, ss):
        st = n * 4 + ss
        ps = ps_sc.tile([P, 1024], F32, tag="sc", name="ps")[:, 0:512]
        for k in range(KD):
            nc.tensor.matmul(ps[:], xts[n][:, k, ts(ss, P)], wv[:, k, :],
                             start=(k == 0), stop=(k == KD - 1))
        nc.vector.tensor_copy(vaug[:, st, :, HD:],
                              ps.rearrange("p (h d) -> p h d", h=GH))

    outTs = [None] * NJ

    def _emit_scores(l, j, i):
        """Score matmuls + exp + boundary mask for (pair l, column j, tile i).
        Returns (pt, off) for the matching PV step."""
        t = i - 4 * j  # >=0 -> diagonal boundary tile
        off = 128 * t if t > 0 else 0
        sc = ps_sc.tile([P, 1024], F32, tag="sc", name="sc")
        scv = sc.rearrange("p (u f) -> p u f", u=2)
        # fp8 DoubleRow at K=64: both DoubleRow k-tiles read the SAME
        # data via a stride-0 broadcast (out = 2*k.q, folded into ESCALE)
        nc.tensor.matmul(
            sc[:, off:512],
            qk8[0:64, 4 + l:5 + l, ts(i, P)].to_broadcast((64, 2, P)),
            qk8[0:64, l:l + 1,
                ds(j * 512 + off, 512 - off)].to_broadcast(
                    (64, 2, 512 - off)),
            start=True, stop=True, perf_mode=DR)
        nc.tensor.matmul(
            sc[:, 512 + off:1024],
            qk8[64:128, 4 + l:5 + l, ts(i, P)].to_broadcast((64, 2, P)),
            qk8[64:128, l:l + 1,
                ds(j * 512 + off, 512 - off)].to_broadcast(
                    (64, 2, 512 - off)),
            start=True, stop=True, perf_mode=DR)
        pt = pt_pool.tile([P, 1024], BF16, tag="pt", name="pt")
        ptv = pt.rearrange("p (u f) -> p u f", u=2)
        nc.scalar.activation(ptv[:, :, off:512], scv[:, :, off:512],
                             EXP, scale=ESCALE)
        if t >= 0:  # causal mask on the boundary 128-col block
            nc.vector.tensor_tensor(
                ptv[:, :, off:off + P], ptv[:, :, off:off + P],
                mask[:, None, :].to_broadcast((P, 2, P)),
                mybir.AluOpType.mult)
        return pt, off

    def _normalize(l, j, pv):
        outT = outTs[j]
        for hh in (0, 1):
            half = pv[:, 512 * hh:512 * (hh + 1)]
            rec = sm.tile([HD, 512], F32, tag="rec", name="rec")
            nc.vector.reciprocal_approx_fast(rec[:], half[0:HD, :])
            nc.vector.tensor_tensor(outT[hh * HD:(hh + 1) * HD, l, :],
                                    half[HD:2 * HD, :], rec[:],
                                    mybir.AluOpType.mult)

    def proj_col_chain(j, m):
        ps = ps_sc.tile([P, 1024], F32, tag="sc", name="ps")[:, 0:512]
        for k in range(KP):
            nc.tensor.matmul(ps[:], wp[:, k, ts(m, P)], outTs[j][:, k, :],
                             start=(k == 0), stop=(k == KP - 1))
        yt = yout.tile([P, 512], BF16, tag="yt", name="yt")
        nc.vector.tensor_copy(yt[:], ps[:])
        # alternate DGE queues so the final column's stores drain in parallel
        eng = nc.sync if m % 2 == 0 else nc.gpsimd
        eng.dma_start(yT_t[:, m, ts(j, 512)], yt[:])

    def proj(j):
        for m in range(8):
            proj_col_chain(j, m)

    class Pacer:
        # Bresenham-paced emission of filler matmul chains between
        # attention iterations, to keep the PE dense (HAM stays warm).
        # Urgent thunks (deferred softmax normalizes) fire one per tick
        # ahead of the paced stream so DVE recips interleave with, not
        # ahead of, the next pair's mask multiplies.
        def __init__(self, thunks, total_ticks):
            self.thunks = list(thunks)
            self.total = max(1, total_ticks)
            self.ticks = 0
            self.fired = 0
            self.urgent = []

        def inject(self, thunks):
            self.urgent.extend(thunks)

        def tick(self):
            self.ticks += 1
            if self.urgent:
                self.urgent.pop(0)()
                return
            while (self.fired < len(self.thunks)
                   and self.fired * self.total < self.ticks * len(self.thunks)):
                self.thunks[self.fired]()
                self.fired += 1

        def flush(self):
            for t in self.urgent:
                t()
            self.urgent = []
            while self.fired < len(self.thunks):
                self.thunks[self.fired]()
                self.fired += 1

    # prelude DMAs: wqk8 k-pairs + x8 k-pairs interleaved on the sync queue
    # (the qk chains' critical path); wv + bf16 x on the gpsimd DGE queue in
    # parallel; wp on the vector queue (needed last, at proj time).
    xt0 = xin.tile([P, KD, 512], BF16, tag="xt", name="xt")
    x8t0 = [x8in.tile([P, 2, 512], FP8, tag="x8t", name="x8t")
            for _ in range(KD // 2)]
    for kk in range(KD // 2):
        nc.sync.dma_start(wqk8s[kk][:], wqk8_t[:, 2 * kk:2 * kk + 2, :])
        nc.sync.dma_start(x8t0[kk][:], xT8_t[:, 2 * kk:2 * kk + 2, ts(0, 512)])
    for k in range(KD):
        nc.gpsimd.dma_start(wv[:, k, :], wv_t[:, k, :])
        nc.gpsimd.dma_start(xt0[:, k, :], xT_t[:, k, ts(0, 512)])
    xts[0] = xt0
    x8ts[0] = x8t0
    nc.scalar.dma_start(wp[:], wp_t)

    # prelude: QKV for the first s-block
    for m in range(8):
        qkv_qk_chain(0, m)
    for ss in range(4):
        qkv_v_chain(0, ss)

    # Flattened, software-pipelined attention stream across ALL columns.
    # Filler allocation: qkv(j+1) is pinned to column j (needed at column
    # j+1); the proj chains are all deferred to the LAST column, which has
    # 40% of the attention iterations (and hence PE slack) but no qkv work.
    for j in range(NJ):
        outTs[j] = ot_pool.tile([P, KP, 512], F32R, tag="outT", name="outT")
    pacers = []
    for j in range(NJ):
        thunks = []
        if j + 1 < NJ:
            for m in range(8):
                thunks.append(lambda n=j + 1, m=m: qkv_qk_chain(n, m))
            for ss in range(4):
                thunks.append(lambda n=j + 1, ss=ss: qkv_v_chain(n, ss))
        else:
            for jj in range(NJ - 1):
                for m in range(8):
                    thunks.append(lambda jj=jj, m=m: proj_col_chain(jj, m))
        pacers.append(Pacer(thunks, 4 * 4 * (j + 1)))

    all_items = [(j, l, i) for j in range(NJ) for l in range(4)
                 for i in range(4 * (j + 1))]
    load_x(1)
    pvs = {}

    def fire_pv(j, l, i, pt, off):
        imax = 4 * (j + 1)
        if i == 0:
            pvs[(j, l)] = ps_pv.tile([P, 1024], F32, tag="pv", name="pv")
        pv = pvs[(j, l)]
        nc.tensor.matmul(pv[:, off:512], vaug[:, i, 2 * l, :],
                         pt[:, off:512],
                         start=(i == 0), stop=(i == imax - 1))
        nc.tensor.matmul(pv[:, 512 + off:1024], vaug[:, i, 2 * l + 1, :],
                         pt[:, 512 + off:1024],
                         start=(i == 0), stop=(i == imax - 1))
        if i == imax - 1:
            _normalize(l, j, pvs.pop((j, l)))

    # PV runs LAG items behind score emission so it never consumes a pt that
    # the ACT/DVE side finished only nanoseconds earlier (full SBUF-access
    # latency exposure); the extra stage costs nothing but pt pool depth.
    LAG = 3
    pending = deque()
    j0, l0, i0 = all_items[0]
    pending.append((j0, l0, i0) + _emit_scores(l0, j0, i0))
    for idx, (j, l, i) in enumerate(all_items):
        if idx + 1 < len(all_items):
            jn, ln, i_n = all_items[idx + 1]
            if jn != j:
                # column boundary: the next column's scores read qk8 written
                # by this column's qkv chains -- flush them FIRST so the
                # in-order PE queue never waits on work queued behind it.
                pacers[j].flush()
                if jn + 1 < NJ:
                    load_x(jn + 1)
            pending.append((jn, ln, i_n) + _emit_scores(ln, jn, i_n))
        pacers[j].tick()
        while len(pending) > LAG:
            fire_pv(*pending.popleft())
    while pending:
        fire_pv(*pending.popleft())
    pacers[NJ - 1].flush()
    proj(NJ - 1)


_NC = None


def build_nc():
    global _NC
    if _NC is not None:
        return _NC
    nc = bacc.Bacc("TRN2", target_bir_lowering=False, debug=False)
    xT = nc.dram_tensor("xT", [D, S], BF16, kind="ExternalInput")
    xT8 = nc.dram_tensor("xT8", [D, S], FP8, kind="ExternalInput")
    wqk8T = nc.dram_tensor("wqk8T", [D, EQK], FP8, kind="ExternalInput")
    wvT = nc.dram_tensor("wvT", [D, DS], BF16, kind="ExternalInput")
    wprojT = nc.dram_tensor("wprojT", [DS, D], F32R, kind="ExternalInput")
    yT = nc.dram_tensor("yT", [D, S], BF16, kind="ExternalOutput")
    with tile.TileContext(nc) as tc:
        _emit(tc, xT.ap(), xT8.ap(), wqk8T.ap(), wvT.ap(), wprojT.ap(),
              yT.ap())
    nc.compile()
    _NC = nc
    return nc


def make_in_maps(x, w_attn, w_proj):
    x = np.ascontiguousarray(np.asarray(x, dtype=np.float32))
    w_attn = np.asarray(w_attn, dtype=np.float32)
    w_proj = np.asarray(w_proj, dtype=np.float32)
    in_maps = []
    for c in range(8):
        b, g = divmod(c, 2)
        rows = slice(g * DS, (g + 1) * DS)
        wq_c = w_attn[0 * D:1 * D][rows] * SW           # [512, 1024]
        wk_c = w_attn[1 * D:2 * D][rows] * SW
        wqk8_c = np.concatenate([wq_c, wk_c], axis=0)   # [1024, 1024]
        wv_c = w_attn[2 * D:3 * D][rows]                # [512, 1024]
        xTb = np.ascontiguousarray(x[b].T)
        in_maps.append({
            "xT": xTb.astype(ml_dtypes.bfloat16),
            "xT8": (xTb * SX).astype(ml_dtypes.float8_e4m3),
            "wqk8T": np.ascontiguousarray(wqk8_c.T).astype(
                ml_dtypes.float8_e4m3),
            "wvT": np.ascontiguousarray(wv_c.T).astype(ml_dtypes.bfloat16),
            "wprojT": np.ascontiguousarray(w_proj[:, rows].T),  # [512, 1024]
        })
    return in_maps


def gather(results):
    y = np.empty((B, S, D), dtype=np.float32)
    for b in range(B):
        yT = (results[2 * b]["yT"].astype(np.float32)
              + results[2 * b + 1]["yT"].astype(np.float32))
        y[b] = yT.T
    return y


def run(x, w_attn, w_proj, trace=False, tmpdir=None):
    nc = build_nc()
    in_maps = make_in_maps(x, w_attn, w_proj)
    res = run_bass_kernel_spmd(nc, in_maps, list(range(8)),
                               trace=trace, tmpdir=tmpdir)
    return gather(res.results), res


def kernel(x, w_attn, w_proj):
    y, _ = run(x, w_attn, w_proj)
    return y


# revision 52
# speedup vs baseline: 1.3140x; 1.0828x over previous
"""Causal self-attention (B=4, S=2048, D=1024, H=16, HD=64) on 8 trn2 cores.

Sharding: core c handles batch b = c//2 and head-group g = c%2 (8 heads).
Each core computes its 8 heads' attention plus the partial output
projection over its d-slice; the host adds the two partial y's per batch.

Device layout is fully transposed ([feature, seq]) so every matmul
contraction lands on the partition dim with no on-device transposes:
  q/k   = wqk8^T @ x8         (fp8e4 DoubleRow, 2x PE rate, fp32 psum)
  v     = x^T @ wv            (bf16)
  scoresT[s_k, s_q] = k8^T @ q8   (fp8e4 DoubleRow at K=64: both DR
                                   k-tiles read the same data via a
                                   stride-0 broadcast; the 2x product
                                   is folded into ESCALE)
  pT = exp(scoresT/(8*256))   (ACT, bf16 out; triangular mask on boundary)
  pv[128, s_q] = v_aug^T @ pT (bf16; rows 0-63 = ones block -> replicated
                               softmax denominators, rows 64-127 = out)
  yT = wprojT^T @ (outT / denom)               (float32r)
Scales: x8 = 4x, wqk8 = 4w  ->  q8 = 16q, scores psum = 256*s; the exp
scale folds the 1/256 back out.  v/proj stay bf16/f32r so the softmax
output path keeps full precision.
QKV(n=j+1) and proj(j-1) matmul chains are interleaved into attention
column j so the PE never idles long enough for HAM to re-throttle.
"""

from collections import deque
from contextlib import ExitStack

import ml_dtypes
import numpy as np

import concourse.bacc as bacc
import concourse.mybir as mybir
import concourse.tile as tile
from concourse._compat import with_exitstack
from concourse.bass import ds, ts  # noqa: E402
from concourse.bass_utils import run_bass_kernel_spmd
from concourse.masks import make_upper_triangular

B, S, D = 4, 2048, 1024
H, HD = 16, 64
P = 128
GH = 8            # heads per core
DS = GH * HD      # 512, d-slice per core
EQK = 2 * DS      # 1024 q+k features per core
KD = D // P       # 8 contraction subtiles for qkv
KP = DS // P      # 4 contraction subtiles for proj
NJ = S // 512     # 4 s_q tiles of 512
NST = S // P      # 16 s_k tiles of 128
F32 = mybir.dt.float32
F32R = mybir.dt.float32r
BF16 = mybir.dt.bfloat16
FP8 = mybir.dt.float8e4
EXP = mybir.ActivationFunctionType.Exp
DR = mybir.MatmulPerfMode.DoubleRow

SX = 4.0          # host scale on x8
SW = 4.0          # host scale on wqk8
# exp scale absorbing fp8 scaling; extra /2 because the score matmul feeds
# the same data through both DoubleRow k-tiles (stride-0 broadcast), which
# doubles the accumulated product.
ESCALE = 0.125 / (SX * SW) ** 2 / 2


@with_exitstack
def _emit(ctx: ExitStack, tc: tile.TileContext, xT, xT8, wqk8T, wvT, wprojT,
          yT):
    nc = tc.nc

    xT_t = xT.rearrange("(ko ki) s -> ki ko s", ki=P)      # [128, 8, 2048]
    xT8_t = xT8.rearrange("(ko ki) s -> ki ko s", ki=P)    # [128, 8, 2048]
    wqk8_t = wqk8T.rearrange("(ko ki) e -> ki ko e", ki=P)  # [128, 8, 1024]
    wv_t = wvT.rearrange("(ko ki) e -> ki ko e", ki=P)     # [128, 8, 512]
    wp_t = wprojT.rearrange("(ko ki) e -> ki ko e", ki=P)  # [128, 4, 1024]
    yT_t = yT.rearrange("(mo mi) s -> mi mo s", mi=P)      # [128, 8, 2048]

    const = ctx.enter_context(tc.tile_pool(name="const", bufs=1))
    qk_pool = ctx.enter_context(tc.tile_pool(name="qkp", bufs=1))
    big = ctx.enter_context(tc.tile_pool(name="big", bufs=1))
    pt_pool = ctx.enter_context(tc.tile_pool(name="ptp", bufs=8))
    xin = ctx.enter_context(tc.tile_pool(name="xin", bufs=2))
    x8in = ctx.enter_context(tc.tile_pool(name="x8in", bufs=8))
    ot_pool = ctx.enter_context(tc.tile_pool(name="otp", bufs=4))
    sm = ctx.enter_context(tc.tile_pool(name="sm", bufs=4))
    yout = ctx.enter_context(tc.tile_pool(name="yo", bufs=3))
    ps_sc = ctx.enter_context(tc.tile_pool(name="ps_sc", bufs=2, space="PSUM"))
    ps_pv = ctx.enter_context(tc.tile_pool(name="ps_pv", bufs=2, space="PSUM"))

    wp = const.tile([P, KP, D], F32R)
    # wqk8 split at DoubleRow k-PAIR granularity so the first QKV chain's
    # dependencies resolve per 256-KB slice instead of per whole tensor
    wqk8s = [const.tile([P, 2, EQK], FP8, name=f"wqk8_{kk}")
             for kk in range(KD // 2)]
    wv = const.tile([P, KD, DS], BF16)
    mask = const.tile([P, P], BF16)
    make_upper_triangular(nc, mask[:], val=1.0, diag=True)

    # qkT: e-tiles 0-3 = q head pairs, 4-7 = k head pairs; [e_in, tile, s]
    # fp8: the clock governor throttles on total PE duty, so halving score
    # row-work via DoubleRow keeps the whole chip's clocks up.
    qk8 = qk_pool.tile([P, 8, S], FP8)
    # v natural layout + 64-wide ones block per head: [s_in, s_tile, head, 128]
    # Ones block FIRST: PV psum rows 0-63 = denom copies, 64-127 = out.
    # (reciprocal_approx_fast drops the partition offset of its input AP, so
    # the denominators must sit at partition 0.)
    vaug = big.tile([P, NST, GH, 2 * HD], BF16)
    nc.gpsimd.memset(vaug[:, :, :, 0:HD], 1.0)

    xts = [None] * NJ
    x8ts = [None] * NJ

    def load_x(n):
        # x loads go out on the gpsimd DGE queue so they don't queue behind
        # the y stores on the sync queue
        xt = xin.tile([P, KD, 512], BF16, tag="xt", name="xt")
        nc.gpsimd.dma_start(xt[:], xT_t[:, :, ts(n, 512)])
        xts[n] = xt
        x8t = [x8in.tile([P, 2, 512], FP8, tag="x8t", name="x8t")
               for _ in range(KD // 2)]
        for kk in range(KD // 2):
            nc.gpsimd.dma_start(x8t[kk][:],
                                xT8_t[:, 2 * kk:2 * kk + 2, ts(n, 512)])
        x8ts[n] = x8t

    def qkv_qk_chain(n, m):
        ps = ps_pv.tile([P, 1024], F32, tag="pv", name="ps")[:, 0:512]
        for kk in range(KD // 2):
            nc.tensor.matmul(ps[:], wqk8s[kk][:, :, ts(m, P)],
                             x8ts[n][kk][:],
                             start=(kk == 0), stop=(kk == KD // 2 - 1),
                             perf_mode=DR)
        nc.vector.tensor_copy(qk8[:, m, ts(n, 512)], ps[:])

    def qkv_v_chain(n, ss):
        st = n * 4 + ss
        ps = ps_pv.tile([P, 1024], F32, tag="pv", name="ps")[:, 0:512]
        for k in range(KD):
            nc.tensor.matmul(ps[:], xts[n][:, k, ts(ss, P)], wv[:, k, :],
                             start=(k == 0), stop=(k == KD - 1))
        nc.vector.tensor_copy(vaug[:, st, :, HD:],
                              ps.rearrange("p (h d) -> p h d", h=GH))

    outTs = [None] * NJ

    def _emit_scores(l, j, i):
        """Score matmuls + exp + boundary mask for (pair l, column j, tile i).
        Returns (pt, off) for the matching PV step."""
        t = i - 4 * j  # >=0 -> diagonal boundary tile
        off = 128 * t if t > 0 else 0
        sc = ps_sc.tile([P, 1024], F32, tag="sc", name="sc")
        scv = sc.rearrange("p (u f) -> p u f", u=2)
        # fp8 DoubleRow at K=64: both DoubleRow k-tiles read the SAME
        # data via a stride-0 broadcast (out = 2*k.q, folded into ESCALE)
        nc.tensor.matmul(
            sc[:, off:512],
            qk8[0:64, 4 + l:5 + l, ts(i, P)].to_broadcast((64, 2, P)),
            qk8[0:64, l:l + 1,
                ds(j * 512 + off, 512 - off)].to_broadcast(
                    (64, 2, 512 - off)),
            start=True, stop=True, perf_mode=DR)
        nc.tensor.matmul(
            sc[:, 512 + off:1024],
            qk8[64:128, 4 + l:5 + l, ts(i, P)].to_broadcast((64, 2, P)),
            qk8[64:128, l:l + 1,
                ds(j * 512 + off, 512 - off)].to_broadcast(
                    (64, 2, 512 - off)),
            start=True, stop=True, perf_mode=DR)
        pt = pt_pool.tile([P, 1024], BF16, tag="pt", name="pt")
        ptv = pt.rearrange("p (u f) -> p u f", u=2)
        nc.scalar.activation(ptv[:, :, off:512], scv[:, :, off:512],
                             EXP, scale=ESCALE)
        if t >= 0:  # causal mask on the boundary 128-col block
            nc.vector.tensor_tensor(
                ptv[:, :, off:off + P], ptv[:, :, off:off + P],
                mask[:, None, :].to_broadcast((P, 2, P)),
                mybir.AluOpType.mult)
        return pt, off

    def _normalize(l, j, pv):
        outT = outTs[j]
        for hh in (0, 1):
            half = pv[:, 512 * hh:512 * (hh + 1)]
            rec = sm.tile([HD, 512], F32, tag="rec", name="rec")
            nc.vector.reciprocal_approx_fast(rec[:], half[0:HD, :])
            nc.vector.tensor_tensor(outT[hh * HD:(hh + 1) * HD, l, :],
                                    half[HD:2 * HD, :], rec[:],
                                    mybir.AluOpType.mult)

    def proj_col_chain(j, m):
        ps = ps_pv.tile([P, 1024], F32, tag="pv", name="ps")[:, 0:512]
        for k in range(KP):
            nc.tensor.matmul(ps[:], wp[:, k, ts(m, P)], outTs[j][:, k, :],
                             start=(k == 0), stop=(k == KP - 1))
        yt = yout.tile([P, 512], BF16, tag="yt", name="yt")
        nc.vector.tensor_copy(yt[:], ps[:])
        # alternate DGE queues so the final column's stores drain in parallel
        eng = nc.sync if m % 2 == 0 else nc.gpsimd
        eng.dma_start(yT_t[:, m, ts(j, 512)], yt[:])

    def proj(j):
        for m in range(8):
            proj_col_chain(j, m)

    class Pacer:
        # Bresenham-paced emission of filler matmul chains between
        # attention iterations, to keep the PE dense (HAM stays warm).
        # Urgent thunks (deferred softmax normalizes) fire one per tick
        # ahead of the paced stream so DVE recips interleave with, not
        # ahead of, the next pair's mask multiplies.
        def __init__(self, thunks, total_ticks):
            self.thunks = list(thunks)
            self.total = max(1, total_ticks)
            self.ticks = 0
            self.fired = 0
            self.urgent = []

        def inject(self, thunks):
            self.urgent.extend(thunks)

        def tick(self):
            self.ticks += 1
            if self.urgent:
                self.urgent.pop(0)()
                return
            while (self.fired < len(self.thunks)
                   and self.fired * self.total < self.ticks * len(self.thunks)):
                self.thunks[self.fired]()
                self.fired += 1

        def flush(self):
            for t in self.urgent:
                t()
            self.urgent = []
            while self.fired < len(self.thunks):
                self.thunks[self.fired]()
                self.fired += 1

    # prelude DMAs: wqk8 k-pairs + x8 k-pairs interleaved on the sync queue
    # (the qk chains' critical path); wv + bf16 x on the gpsimd DGE queue in
    # parallel; wp on the vector queue (needed last, at proj time).
    xt0 = xin.tile([P, KD, 512], BF16, tag="xt", name="xt")
    x8t0 = [x8in.tile([P, 2, 512], FP8, tag="x8t", name="x8t")
            for _ in range(KD // 2)]
    for kk in range(KD // 2):
        nc.sync.dma_start(wqk8s[kk][:], wqk8_t[:, 2 * kk:2 * kk + 2, :])
        nc.sync.dma_start(x8t0[kk][:], xT8_t[:, 2 * kk:2 * kk + 2, ts(0, 512)])
    for k in range(KD):
        nc.gpsimd.dma_start(wv[:, k, :], wv_t[:, k, :])
        nc.gpsimd.dma_start(xt0[:, k, :], xT_t[:, k, ts(0, 512)])
    xts[0] = xt0
    x8ts[0] = x8t0
    nc.scalar.dma_start(wp[:], wp_t)

    # prelude: QKV for the first s-block
    for m in range(8):
        qkv_qk_chain(0, m)
    for ss in range(4):
        qkv_v_chain(0, ss)

    # Flattened, software-pipelined attention stream across ALL columns.
    # Filler allocation: qkv(j+1) is pinned to column j (needed at column
    # j+1); the proj chains are all deferred to the LAST column, which has
    # 40% of the attention iterations (and hence PE slack) but no qkv work.
    for j in range(NJ):
        outTs[j] = ot_pool.tile([P, KP, 512], F32R, tag="outT", name="outT")
    pacers = []
    for j in range(NJ):
        thunks = []
        if j + 1 < NJ:
            for m in range(8):
                thunks.append(lambda n=j + 1, m=m: qkv_qk_chain(n, m))
            for ss in range(4):
                thunks.append(lambda n=j + 1, ss=ss: qkv_v_chain(n, ss))
        else:
            for jj in range(NJ - 1):
                for m in range(8):
                    thunks.append(lambda jj=jj, m=m: proj_col_chain(jj, m))
        pacers.append(Pacer(thunks, 4 * 4 * (j + 1)))

    all_items = [(j, l, i) for j in range(NJ) for l in range(4)
                 for i in range(4 * (j + 1))]
    load_x(1)
    pvs = {}

    def fire_pv(j, l, i, pt, off):
        imax = 4 * (j + 1)
        if i == 0:
            pvs[(j, l)] = ps_pv.tile([P, 1024], F32, tag="pv", name="pv")
        pv = pvs[(j, l)]
        nc.tensor.matmul(pv[:, off:512], vaug[:, i, 2 * l, :],
                         pt[:, off:512],
                         start=(i == 0), stop=(i == imax - 1))
        nc.tensor.matmul(pv[:, 512 + off:1024], vaug[:, i, 2 * l + 1, :],
                         pt[:, 512 + off:1024],
                         start=(i == 0), stop=(i == imax - 1))
        if i == imax - 1:
            _normalize(l, j, pvs.pop((j, l)))

    # PV runs LAG items behind score emission so it never consumes a pt that
    # the ACT/DVE side finished only nanoseconds earlier (full SBUF-access
    # latency exposure); the extra stage costs nothing but pt pool depth.
    LAG = 3
    pending = deque()
    j0, l0, i0 = all_items[0]
    pending.append((j0, l0, i0) + _emit_scores(l0, j0, i0))
    for idx, (j, l, i) in enumerate(all_items):
        if idx + 1 < len(all_items):
            jn, ln, i_n = all_items[idx + 1]
            if jn != j:
                # column boundary: the next column's scores read qk8 written
                # by this column's qkv chains -- flush them FIRST so the
                # in-order PE queue never waits on work queued behind it.
                pacers[j].flush()
                if jn + 1 < NJ:
                    load_x(jn + 1)
            pending.append((jn, ln, i_n) + _emit_scores(ln, jn, i_n))
        pacers[j].tick()
        while len(pending) > LAG:
            fire_pv(*pending.popleft())
    while pending:
        fire_pv(*pending.popleft())
    pacers[NJ - 1].flush()
    proj(NJ - 1)


_NC = None


def build_nc():
    global _NC
    if _NC is not None:
        return _NC
    nc = bacc.Bacc("TRN2", target_bir_lowering=False, debug=False)
    xT = nc.dram_tensor("xT", [D, S], BF16, kind="ExternalInput")
    xT8 = nc.dram_tensor("xT8", [D, S], FP8, kind="ExternalInput")
    wqk8T = nc.dram_tensor("wqk8T", [D, EQK], FP8, kind="ExternalInput")
    wvT = nc.dram_tensor("wvT", [D, DS], BF16, kind="ExternalInput")
    wprojT = nc.dram_tensor("wprojT", [DS, D], F32R, kind="ExternalInput")
    yT = nc.dram_tensor("yT", [D, S], BF16, kind="ExternalOutput")
    with tile.TileContext(nc) as tc:
        _emit(tc, xT.ap(), xT8.ap(), wqk8T.ap(), wvT.ap(), wprojT.ap(),
              yT.ap())
    nc.compile()
    _NC = nc
    return nc


def make_in_maps(x, w_attn, w_proj):
    x = np.ascontiguousarray(np.asarray(x, dtype=np.float32))
    w_attn = np.asarray(w_attn, dtype=np.float32)
    w_proj = np.asarray(w_proj, dtype=np.float32)
    in_maps = []
    for c in range(8):
        b, g = divmod(c, 2)
        rows = slice(g * DS, (g + 1) * DS)
        wq_c = w_attn[0 * D:1 * D][rows] * SW           # [512, 1024]
        wk_c = w_attn[1 * D:2 * D][rows] * SW
        wqk8_c = np.concatenate([wq_c, wk_c], axis=0)   # [1024, 1024]
        wv_c = w_attn[2 * D:3 * D][rows]                # [512, 1024]
        xTb = np.ascontiguousarray(x[b].T)
        in_maps.append({
            "xT": xTb.astype(ml_dtypes.bfloat16),
            "xT8": (xTb * SX).astype(ml_dtypes.float8_e4m3),
            "wqk8T": np.ascontiguousarray(wqk8_c.T).astype(
                ml_dtypes.float8_e4m3),
            "wvT": np.ascontiguousarray(wv_c.T).astype(ml_dtypes.bfloat16),
            "wprojT": np.ascontiguousarray(w_proj[:, rows].T),  # [512, 1024]
        })
    return in_maps


def gather(results):
    y = np.empty((B, S, D), dtype=np.float32)
    for b in range(B):
        yT = (results[2 * b]["yT"].astype(np.float32)
              + results[2 * b + 1]["yT"].astype(np.float32))
        y[b] = yT.T
    return y


def run(x, w_attn, w_proj, trace=False, tmpdir=None):
    nc = build_nc()
    in_maps = make_in_maps(x, w_attn, w_proj)
    res = run_bass_kernel_spmd(nc, in_maps, list(range(8)),
                               trace=trace, tmpdir=tmpdir)
    return gather(res.results), res


def kernel(x, w_attn, w_proj):
    y, _ = run(x, w_attn, w_proj)
    return y


# revision 55
# speedup vs baseline: 1.3548x; 1.0311x over previous
"""Causal self-attention (B=4, S=2048, D=1024, H=16, HD=64) on 8 trn2 cores.

Sharding: core c handles batch b = c//2 and head-group g = c%2 (8 heads).
Each core computes its 8 heads' attention plus the partial output
projection over its d-slice; the host adds the two partial y's per batch.

Device layout is fully transposed ([feature, seq]) so every matmul
contraction lands on the partition dim with no on-device transposes:
  q/k   = wqk8^T @ x8         (fp8e4 DoubleRow, 2x PE rate, fp32 psum)
  v     = x^T @ wv            (bf16)
  scoresT[s_k, s_q] = k8^T @ q8   (fp8e4 DoubleRow at K=64: both DR
                                   k-tiles read the same data via a
                                   stride-0 broadcast; the 2x product
                                   is folded into ESCALE)
  pT = exp(scoresT/(8*256))   (ACT, bf16 out; triangular mask on boundary)
  pv[128, s_q] = v_aug^T @ pT (bf16; rows 0-63 = ones block -> replicated
                               softmax denominators, rows 64-127 = out)
  yT = wprojT^T @ (outT / denom)               (float32r)
Scales: x8 = 4x, wqk8 = 4w  ->  q8 = 16q, scores psum = 256*s; the exp
scale folds the 1/256 back out.  v/proj stay bf16/f32r so the softmax
output path keeps full precision.
QKV(n=j+1) and proj(j-1) matmul chains are interleaved into attention
column j so the PE never idles long enough for HAM to re-throttle.
"""

from collections import deque
from contextlib import ExitStack

import ml_dtypes
import numpy as np

import concourse.bacc as bacc
import concourse.mybir as mybir
import concourse.tile as tile
from concourse._compat import with_exitstack
from concourse.bass import ds, ts  # noqa: E402
from concourse.bass_utils import run_bass_kernel_spmd
from concourse.masks import make_upper_triangular

B, S, D = 4, 2048, 1024
H, HD = 16, 64
P = 128
GH = 8            # heads per core
DS = GH * HD      # 512, d-slice per core
EQK = 2 * DS      # 1024 q+k features per core
KD = D // P       # 8 contraction subtiles for qkv
KP = DS // P      # 4 contraction subtiles for proj
NJ = S // 512     # 4 s_q tiles of 512
NST = S // P      # 16 s_k tiles of 128
F32 = mybir.dt.float32
F32R = mybir.dt.float32r
BF16 = mybir.dt.bfloat16
FP8 = mybir.dt.float8e4
EXP = mybir.ActivationFunctionType.Exp
DR = mybir.MatmulPerfMode.DoubleRow

SX = 4.0          # host scale on x8
SW = 4.0          # host scale on wqk8
# exp scale absorbing fp8 scaling; extra /2 because the score matmul feeds
# the same data through both DoubleRow k-tiles (stride-0 broadcast), which
# doubles the accumulated product.
ESCALE = 0.125 / (SX * SW) ** 2 / 2


@with_exitstack
def _emit(ctx: ExitStack, tc: tile.TileContext, xT, xT8, wqk8T, wvT, wprojT,
          yT):
    nc = tc.nc

    xT_t = xT.rearrange("(ko ki) s -> ki ko s", ki=P)      # [128, 8, 2048]
    xT8_t = xT8.rearrange("(ko ki) s -> ki ko s", ki=P)    # [128, 8, 2048]
    wqk8_t = wqk8T.rearrange("(ko ki) e -> ki ko e", ki=P)  # [128, 8, 1024]
    wv_t = wvT.rearrange("(ko ki) e -> ki ko e", ki=P)     # [128, 8, 512]
    wp_t = wprojT.rearrange("(ko ki) e -> ki ko e", ki=P)  # [128, 4, 1024]
    yT_t = yT.rearrange("(mo mi) s -> mi mo s", mi=P)      # [128, 8, 2048]

    const = ctx.enter_context(tc.tile_pool(name="const", bufs=1))
    qk_pool = ctx.enter_context(tc.tile_pool(name="qkp", bufs=1))
    big = ctx.enter_context(tc.tile_pool(name="big", bufs=1))
    pt_pool = ctx.enter_context(tc.tile_pool(name="ptp", bufs=8))
    xin = ctx.enter_context(tc.tile_pool(name="xin", bufs=2))
    x8in = ctx.enter_context(tc.tile_pool(name="x8in", bufs=8))
    ot_pool = ctx.enter_context(tc.tile_pool(name="otp", bufs=4))
    sm = ctx.enter_context(tc.tile_pool(name="sm", bufs=4))
    yout = ctx.enter_context(tc.tile_pool(name="yo", bufs=3))
    ps_sc = ctx.enter_context(tc.tile_pool(name="ps_sc", bufs=2, space="PSUM"))
    ps_pv = ctx.enter_context(tc.tile_pool(name="ps_pv", bufs=2, space="PSUM"))

    wp = const.tile([P, KP, D], BF16)
    # wqk8 split at DoubleRow k-PAIR granularity so the first QKV chain's
    # dependencies resolve per 256-KB slice instead of per whole tensor
    wqk8s = [const.tile([P, 2, EQK], FP8, name=f"wqk8_{kk}")
             for kk in range(KD // 2)]
    wv = const.tile([P, KD, DS], BF16)
    mask = const.tile([P, P], BF16)
    make_upper_triangular(nc, mask[:], val=1.0, diag=True)

    # qkT: e-tiles 0-3 = q head pairs, 4-7 = k head pairs; [e_in, tile, s]
    # fp8: the clock governor throttles on total PE duty, so halving score
    # row-work via DoubleRow keeps the whole chip's clocks up.
    qk8 = qk_pool.tile([P, 8, S], FP8)
    # v natural layout + 64-wide ones block per head: [s_in, s_tile, head, 128]
    # Ones block FIRST: PV psum rows 0-63 = denom copies, 64-127 = out.
    # (reciprocal_approx_fast drops the partition offset of its input AP, so
    # the denominators must sit at partition 0.)
    vaug = big.tile([P, NST, GH, 2 * HD], BF16)
    nc.gpsimd.memset(vaug[:, :, :, 0:HD], 1.0)

    xts = [None] * NJ
    x8ts = [None] * NJ

    def load_x(n):
        # x loads go out on the gpsimd DGE queue so they don't queue behind
        # the y stores on the sync queue
        xt = xin.tile([P, KD, 512], BF16, tag="xt", name="xt")
        nc.gpsimd.dma_start(xt[:], xT_t[:, :, ts(n, 512)])
        xts[n] = xt
        x8t = [x8in.tile([P, 2, 512], FP8, tag="x8t", name="x8t")
               for _ in range(KD // 2)]
        for kk in range(KD // 2):
            nc.gpsimd.dma_start(x8t[kk][:],
                                xT8_t[:, 2 * kk:2 * kk + 2, ts(n, 512)])
        x8ts[n] = x8t

    def qkv_qk_chain(n, m):
        ps = ps_pv.tile([P, 1024], F32, tag="pv", name="ps")[:, 0:512]
        for kk in range(KD // 2):
            nc.tensor.matmul(ps[:], wqk8s[kk][:, :, ts(m, P)],
                             x8ts[n][kk][:],
                             start=(kk == 0), stop=(kk == KD // 2 - 1),
                             perf_mode=DR)
        nc.vector.tensor_copy(qk8[:, m, ts(n, 512)], ps[:])

    def qkv_v_chain(n, ss):
        st = n * 4 + ss
        ps = ps_pv.tile([P, 1024], F32, tag="pv", name="ps")[:, 0:512]
        for k in range(KD):
            nc.tensor.matmul(ps[:], xts[n][:, k, ts(ss, P)], wv[:, k, :],
                             start=(k == 0), stop=(k == KD - 1))
        nc.vector.tensor_copy(vaug[:, st, :, HD:],
                              ps.rearrange("p (h d) -> p h d", h=GH))

    outTs = [None] * NJ

    def _emit_scores(l, j, i):
        """Score matmuls + exp + boundary mask for (pair l, column j, tile i).
        Returns (pt, off) for the matching PV step."""
        t = i - 4 * j  # >=0 -> diagonal boundary tile
        off = 128 * t if t > 0 else 0
        sc = ps_sc.tile([P, 1024], F32, tag="sc", name="sc")
        scv = sc.rearrange("p (u f) -> p u f", u=2)
        # fp8 DoubleRow at K=64: both DoubleRow k-tiles read the SAME
        # data via a stride-0 broadcast (out = 2*k.q, folded into ESCALE)
        nc.tensor.matmul(
            sc[:, off:512],
            qk8[0:64, 4 + l:5 + l, ts(i, P)].to_broadcast((64, 2, P)),
            qk8[0:64, l:l + 1,
                ds(j * 512 + off, 512 - off)].to_broadcast(
                    (64, 2, 512 - off)),
            start=True, stop=True, perf_mode=DR)
        nc.tensor.matmul(
            sc[:, 512 + off:1024],
            qk8[64:128, 4 + l:5 + l, ts(i, P)].to_broadcast((64, 2, P)),
            qk8[64:128, l:l + 1,
                ds(j * 512 + off, 512 - off)].to_broadcast(
                    (64, 2, 512 - off)),
            start=True, stop=True, perf_mode=DR)
        pt = pt_pool.tile([P, 1024], BF16, tag="pt", name="pt")
        ptv = pt.rearrange("p (u f) -> p u f", u=2)
        nc.scalar.activation(ptv[:, :, off:512], scv[:, :, off:512],
                             EXP, scale=ESCALE)
        if t >= 0:  # causal mask on the boundary 128-col block
            nc.vector.tensor_tensor(
                ptv[:, :, off:off + P], ptv[:, :, off:off + P],
                mask[:, None, :].to_broadcast((P, 2, P)),
                mybir.AluOpType.mult)
        return pt, off

    def _normalize(l, j, pv):
        outT = outTs[j]
        for hh in (0, 1):
            half = pv[:, 512 * hh:512 * (hh + 1)]
            rec = sm.tile([HD, 512], F32, tag="rec", name="rec")
            nc.vector.reciprocal_approx_fast(rec[:], half[0:HD, :])
            nc.vector.tensor_tensor(outT[hh * HD:(hh + 1) * HD, l, :],
                                    half[HD:2 * HD, :], rec[:],
                                    mybir.AluOpType.mult)

    def proj_col_chain(j, m):
        ps = ps_pv.tile([P, 1024], F32, tag="pv", name="ps")[:, 0:512]
        for k in range(KP):
            nc.tensor.matmul(ps[:], wp[:, k, ts(m, P)], outTs[j][:, k, :],
                             start=(k == 0), stop=(k == KP - 1))
        yt = yout.tile([P, 512], BF16, tag="yt", name="yt")
        nc.vector.tensor_copy(yt[:], ps[:])
        # alternate DGE queues so the final column's stores drain in parallel
        eng = nc.sync if m % 2 == 0 else nc.gpsimd
        eng.dma_start(yT_t[:, m, ts(j, 512)], yt[:])

    def proj(j):
        for m in range(8):
            proj_col_chain(j, m)

    class Pacer:
        # Bresenham-paced emission of filler matmul chains between
        # attention iterations, to keep the PE dense (HAM stays warm).
        # Urgent thunks (deferred softmax normalizes) fire one per tick
        # ahead of the paced stream so DVE recips interleave with, not
        # ahead of, the next pair's mask multiplies.
        def __init__(self, thunks, total_ticks):
            self.thunks = list(thunks)
            self.total = max(1, total_ticks)
            self.ticks = 0
            self.fired = 0
            self.urgent = []

        def inject(self, thunks):
            self.urgent.extend(thunks)

        def tick(self):
            self.ticks += 1
            if self.urgent:
                self.urgent.pop(0)()
                return
            while (self.fired < len(self.thunks)
                   and self.fired * self.total < self.ticks * len(self.thunks)):
                self.thunks[self.fired]()
                self.fired += 1

        def flush(self):
            for t in self.urgent:
                t()
            self.urgent = []
            while self.fired < len(self.thunks):
                self.thunks[self.fired]()
                self.fired += 1

    # prelude DMAs: wqk8 k-pairs + x8 k-pairs interleaved on the sync queue
    # (the qk chains' critical path); wv + bf16 x on the gpsimd DGE queue in
    # parallel; wp on the vector queue (needed last, at proj time).
    xt0 = xin.tile([P, KD, 512], BF16, tag="xt", name="xt")
    x8t0 = [x8in.tile([P, 2, 512], FP8, tag="x8t", name="x8t")
            for _ in range(KD // 2)]
    for kk in range(KD // 2):
        nc.sync.dma_start(wqk8s[kk][:], wqk8_t[:, 2 * kk:2 * kk + 2, :])
        nc.sync.dma_start(x8t0[kk][:], xT8_t[:, 2 * kk:2 * kk + 2, ts(0, 512)])
    for k in range(KD):
        nc.gpsimd.dma_start(wv[:, k, :], wv_t[:, k, :])
        nc.gpsimd.dma_start(xt0[:, k, :], xT_t[:, k, ts(0, 512)])
    xts[0] = xt0
    x8ts[0] = x8t0
    nc.scalar.dma_start(wp[:], wp_t)

    # prelude: QKV for the first s-block
    for m in range(8):
        qkv_qk_chain(0, m)
    for ss in range(4):
        qkv_v_chain(0, ss)

    # Flattened, software-pipelined attention stream across ALL columns.
    # Filler allocation: qkv(j+1) is pinned to column j (needed at column
    # j+1); the proj chains are all deferred to the LAST column, which has
    # 40% of the attention iterations (and hence PE slack) but no qkv work.
    for j in range(NJ):
        outTs[j] = ot_pool.tile([P, KP, 512], BF16, tag="outT", name="outT")
    pacers = []
    for j in range(NJ):
        thunks = []
        if j + 1 < NJ:
            for m in range(8):
                thunks.append(lambda n=j + 1, m=m: qkv_qk_chain(n, m))
            for ss in range(4):
                thunks.append(lambda n=j + 1, ss=ss: qkv_v_chain(n, ss))
        else:
            for jj in range(NJ - 1):
                for m in range(8):
                    thunks.append(lambda jj=jj, m=m: proj_col_chain(jj, m))
        pacers.append(Pacer(thunks, 4 * 4 * (j + 1)))

    all_items = [(j, l, i) for j in range(NJ) for l in range(4)
                 for i in range(4 * (j + 1))]
    load_x(1)
    pvs = {}

    def fire_pv(j, l, i, pt, off):
        imax = 4 * (j + 1)
        if i == 0:
            pvs[(j, l)] = ps_pv.tile([P, 1024], F32, tag="pv", name="pv")
        pv = pvs[(j, l)]
        nc.tensor.matmul(pv[:, off:512], vaug[:, i, 2 * l, :],
                         pt[:, off:512],
                         start=(i == 0), stop=(i == imax - 1))
        nc.tensor.matmul(pv[:, 512 + off:1024], vaug[:, i, 2 * l + 1, :],
                         pt[:, 512 + off:1024],
                         start=(i == 0), stop=(i == imax - 1))
        if i == imax - 1:
            _normalize(l, j, pvs.pop((j, l)))

    # PV runs LAG items behind score emission so it never consumes a pt that
    # the ACT/DVE side finished only nanoseconds earlier (full SBUF-access
    # latency exposure); the extra stage costs nothing but pt pool depth.
    LAG = 3
    pending = deque()
    j0, l0, i0 = all_items[0]
    pending.append((j0, l0, i0) + _emit_scores(l0, j0, i0))
    for idx, (j, l, i) in enumerate(all_items):
        if idx + 1 < len(all_items):
            jn, ln, i_n = all_items[idx + 1]
            if jn != j:
                # column boundary: the next column's scores read qk8 written
                # by this column's qkv chains -- flush them FIRST so the
                # in-order PE queue never waits on work queued behind it.
                pacers[j].flush()
                if jn + 1 < NJ:
                    load_x(jn + 1)
            pending.append((jn, ln, i_n) + _emit_scores(ln, jn, i_n))
        pacers[j].tick()
        while len(pending) > LAG:
            fire_pv(*pending.popleft())
    while pending:
        fire_pv(*pending.popleft())
    pacers[NJ - 1].flush()
    proj(NJ - 1)


_NC = None


def build_nc():
    global _NC
    if _NC is not None:
        return _NC
    nc = bacc.Bacc("TRN2", target_bir_lowering=False, debug=False)
    xT = nc.dram_tensor("xT", [D, S], BF16, kind="ExternalInput")
    xT8 = nc.dram_tensor("xT8", [D, S], FP8, kind="ExternalInput")
    wqk8T = nc.dram_tensor("wqk8T", [D, EQK], FP8, kind="ExternalInput")
    wvT = nc.dram_tensor("wvT", [D, DS], BF16, kind="ExternalInput")
    wprojT = nc.dram_tensor("wprojT", [DS, D], BF16, kind="ExternalInput")
    yT = nc.dram_tensor("yT", [D, S], BF16, kind="ExternalOutput")
    with tile.TileContext(nc) as tc:
        _emit(tc, xT.ap(), xT8.ap(), wqk8T.ap(), wvT.ap(), wprojT.ap(),
              yT.ap())
    nc.compile()
    _NC = nc
    return nc


def make_in_maps(x, w_attn, w_proj):
    x = np.ascontiguousarray(np.asarray(x, dtype=np.float32))
    w_attn = np.asarray(w_attn, dtype=np.float32)
    w_proj = np.asarray(w_proj, dtype=np.float32)
    in_maps = []
    for c in range(8):
        b, g = divmod(c, 2)
        rows = slice(g * DS, (g + 1) * DS)
        wq_c = w_attn[0 * D:1 * D][rows] * SW           # [512, 1024]
        wk_c = w_attn[1 * D:2 * D][rows] * SW
        wqk8_c = np.concatenate([wq_c, wk_c], axis=0)   # [1024, 1024]
        wv_c = w_attn[2 * D:3 * D][rows]                # [512, 1024]
        xTb = np.ascontiguousarray(x[b].T)
        in_maps.append({
            "xT": xTb.astype(ml_dtypes.bfloat16),
            "xT8": (xTb * SX).astype(ml_dtypes.float8_e4m3),
            "wqk8T": np.ascontiguousarray(wqk8_c.T).astype(
                ml_dtypes.float8_e4m3),
            "wvT": np.ascontiguousarray(wv_c.T).astype(ml_dtypes.bfloat16),
            "wprojT": np.ascontiguousarray(w_proj[:, rows].T).astype(
                ml_dtypes.bfloat16),  # [512, 1024]
        })
    return in_maps


def gather(results):
    y = np.empty((B, S, D), dtype=np.float32)
    for b in range(B):
        yT = (results[2 * b]["yT"].astype(np.float32)
              + results[2 * b + 1]["yT"].astype(np.float32))
        y[b] = yT.T
    return y


def run(x, w_attn, w_proj, trace=False, tmpdir=None):
    nc = build_nc()
    in_maps = make_in_maps(x, w_attn, w_proj)
    res = run_bass_kernel_spmd(nc, in_maps, list(range(8)),
                               trace=trace, tmpdir=tmpdir)
    return gather(res.results), res


def kernel(x, w_attn, w_proj):
    y, _ = run(x, w_attn, w_proj)
    return y


# revision 57
# speedup vs baseline: 1.3667x; 1.0087x over previous
"""Causal self-attention (B=4, S=2048, D=1024, H=16, HD=64) on 8 trn2 cores.

Sharding: core c handles batch b = c//2 and head-group g = c%2 (8 heads).
Each core computes its 8 heads' attention plus the partial output
projection over its d-slice; the host adds the two partial y's per batch.

Device layout is fully transposed ([feature, seq]) so every matmul
contraction lands on the partition dim with no on-device transposes:
  q/k   = wqk8^T @ x8         (fp8e4 DoubleRow, 2x PE rate, fp32 psum)
  v     = x^T @ wv            (bf16)
  scoresT[s_k, s_q] = k8^T @ q8   (fp8e4 DoubleRow at K=64: both DR
                                   k-tiles read the same data via a
                                   stride-0 broadcast; the 2x product
                                   is folded into ESCALE)
  pT = exp(scoresT/(8*256))   (ACT, bf16 out; triangular mask on boundary)
  pv[128, s_q] = v_aug^T @ pT (bf16; rows 0-63 = ones block -> replicated
                               softmax denominators, rows 64-127 = out)
  yT = wprojT^T @ (outT / denom)               (bf16, f32 psum)
Scales: x8 = 4x, wqk8 = 4w  ->  q8 = 16q, scores psum = 256*s; the exp
scale folds the 1/256 back out.  v/pv/proj stay bf16 with f32 psum so
the softmax output path keeps full precision.
QKV(n=j+1) and proj(j-1) matmul chains are interleaved into attention
column j so the PE never idles long enough for HAM to re-throttle.
"""

from collections import deque
from contextlib import ExitStack

import ml_dtypes
import numpy as np

import concourse.bacc as bacc
import concourse.mybir as mybir
import concourse.tile as tile
from concourse._compat import with_exitstack
from concourse.bass import ds, ts  # noqa: E402
from concourse.bass_utils import run_bass_kernel_spmd
from concourse.masks import make_upper_triangular

B, S, D = 4, 2048, 1024
H, HD = 16, 64
P = 128
GH = 8            # heads per core
DS = GH * HD      # 512, d-slice per core
EQK = 2 * DS      # 1024 q+k features per core
KD = D // P       # 8 contraction subtiles for qkv
KP = DS // P      # 4 contraction subtiles for proj
NJ = S // 512     # 4 s_q tiles of 512
NST = S // P      # 16 s_k tiles of 128
F32 = mybir.dt.float32
F32R = mybir.dt.float32r
BF16 = mybir.dt.bfloat16
FP8 = mybir.dt.float8e4
EXP = mybir.ActivationFunctionType.Exp
DR = mybir.MatmulPerfMode.DoubleRow

SX = 4.0          # host scale on x8
SW = 4.0          # host scale on wqk8
# exp scale absorbing fp8 scaling; extra /2 because the score matmul feeds
# the same data through both DoubleRow k-tiles (stride-0 broadcast), which
# doubles the accumulated product.
ESCALE = 0.125 / (SX * SW) ** 2 / 2


@with_exitstack
def _emit(ctx: ExitStack, tc: tile.TileContext, xT, xT8, wqk8T, wvT, wprojT,
          yT):
    nc = tc.nc

    xT_t = xT.rearrange("(ko ki) s -> ki ko s", ki=P)      # [128, 8, 2048]
    xT8_t = xT8.rearrange("(ko ki) s -> ki ko s", ki=P)    # [128, 8, 2048]
    wqk8_t = wqk8T.rearrange("(ko ki) e -> ki ko e", ki=P)  # [128, 8, 1024]
    wv_t = wvT.rearrange("(ko ki) e -> ki ko e", ki=P)     # [128, 8, 512]
    wp_t = wprojT.rearrange("(ko ki) e -> ki ko e", ki=P)  # [128, 4, 1024]
    yT_t = yT.rearrange("(mo mi) s -> mi mo s", mi=P)      # [128, 8, 2048]

    const = ctx.enter_context(tc.tile_pool(name="const", bufs=1))
    qk_pool = ctx.enter_context(tc.tile_pool(name="qkp", bufs=1))
    big = ctx.enter_context(tc.tile_pool(name="big", bufs=1))
    pt_pool = ctx.enter_context(tc.tile_pool(name="ptp", bufs=8))
    xin = ctx.enter_context(tc.tile_pool(name="xin", bufs=2))
    x8in = ctx.enter_context(tc.tile_pool(name="x8in", bufs=8))
    ot_pool = ctx.enter_context(tc.tile_pool(name="otp", bufs=4))
    sm = ctx.enter_context(tc.tile_pool(name="sm", bufs=4))
    yout = ctx.enter_context(tc.tile_pool(name="yo", bufs=3))
    ps_sc = ctx.enter_context(tc.tile_pool(name="ps_sc", bufs=2, space="PSUM"))
    ps_pv = ctx.enter_context(tc.tile_pool(name="ps_pv", bufs=2, space="PSUM"))

    wp = const.tile([P, KP, D], BF16)
    # wqk8 split at DoubleRow k-PAIR granularity so the first QKV chain's
    # dependencies resolve per 256-KB slice instead of per whole tensor
    wqk8s = [const.tile([P, 2, EQK], FP8, name=f"wqk8_{kk}")
             for kk in range(KD // 2)]
    wv = const.tile([P, KD, DS], BF16)
    mask = const.tile([P, P], BF16)
    make_upper_triangular(nc, mask[:], val=1.0, diag=True)

    # qkT: e-tiles 0-3 = q head pairs, 4-7 = k head pairs; [e_in, tile, s]
    # fp8: the clock governor throttles on total PE duty, so halving score
    # row-work via DoubleRow keeps the whole chip's clocks up.
    qk8 = qk_pool.tile([P, 8, S], FP8)
    # v natural layout + 64-wide ones block per head: [s_in, s_tile, head, 128]
    # Ones block FIRST: PV psum rows 0-63 = denom copies, 64-127 = out.
    # (reciprocal_approx_fast drops the partition offset of its input AP, so
    # the denominators must sit at partition 0.)
    vaug = big.tile([P, NST, GH, 2 * HD], BF16)
    nc.gpsimd.memset(vaug[:, :, :, 0:HD], 1.0)

    xts = [None] * NJ
    x8ts = [None] * NJ

    def load_x(n):
        # x loads go out on the gpsimd DGE queue so they don't queue behind
        # the y stores on the sync queue
        xt = xin.tile([P, KD, 512], BF16, tag="xt", name="xt")
        nc.gpsimd.dma_start(xt[:], xT_t[:, :, ts(n, 512)])
        xts[n] = xt
        x8t = [x8in.tile([P, 2, 512], FP8, tag="x8t", name="x8t")
               for _ in range(KD // 2)]
        for kk in range(KD // 2):
            nc.gpsimd.dma_start(x8t[kk][:],
                                xT8_t[:, 2 * kk:2 * kk + 2, ts(n, 512)])
        x8ts[n] = x8t

    def qkv_qk_chain(n, m):
        ps = ps_pv.tile([P, 1024], F32, tag="pv", name="ps")[:, 0:512]
        for kk in range(KD // 2):
            nc.tensor.matmul(ps[:], wqk8s[kk][:, :, ts(m, P)],
                             x8ts[n][kk][:],
                             start=(kk == 0), stop=(kk == KD // 2 - 1),
                             perf_mode=DR)
        nc.vector.tensor_copy(qk8[:, m, ts(n, 512)], ps[:])

    def qkv_v_chain(n, ss):
        st = n * 4 + ss
        ps = ps_pv.tile([P, 1024], F32, tag="pv", name="ps")[:, 0:512]
        for k in range(KD):
            nc.tensor.matmul(ps[:], xts[n][:, k, ts(ss, P)], wv[:, k, :],
                             start=(k == 0), stop=(k == KD - 1))
        nc.vector.tensor_copy(vaug[:, st, :, HD:],
                              ps.rearrange("p (h d) -> p h d", h=GH))

    outTs = [None] * NJ

    def _emit_scores(l, j, i):
        """Score matmuls + exp + boundary mask for (pair l, column j, tile i).
        Returns (pt, off) for the matching PV step."""
        t = i - 4 * j  # >=0 -> diagonal boundary tile
        off = 128 * t if t > 0 else 0
        sc = ps_sc.tile([P, 1024], F32, tag="sc", name="sc")
        scv = sc.rearrange("p (u f) -> p u f", u=2)
        # fp8 DoubleRow at K=64: both DoubleRow k-tiles read the SAME
        # data via a stride-0 broadcast (out = 2*k.q, folded into ESCALE)
        nc.tensor.matmul(
            sc[:, off:512],
            qk8[0:64, 4 + l:5 + l, ts(i, P)].to_broadcast((64, 2, P)),
            qk8[0:64, l:l + 1,
                ds(j * 512 + off, 512 - off)].to_broadcast(
                    (64, 2, 512 - off)),
            start=True, stop=True, perf_mode=DR)
        nc.tensor.matmul(
            sc[:, 512 + off:1024],
            qk8[64:128, 4 + l:5 + l, ts(i, P)].to_broadcast((64, 2, P)),
            qk8[64:128, l:l + 1,
                ds(j * 512 + off, 512 - off)].to_broadcast(
                    (64, 2, 512 - off)),
            start=True, stop=True, perf_mode=DR)
        pt = pt_pool.tile([P, 1024], BF16, tag="pt", name="pt")
        ptv = pt.rearrange("p (u f) -> p u f", u=2)
        nc.scalar.activation(ptv[:, :, off:512], scv[:, :, off:512],
                             EXP, scale=ESCALE)
        if t >= 0:  # causal mask on the boundary 128-col block
            nc.vector.tensor_tensor(
                ptv[:, :, off:off + P], ptv[:, :, off:off + P],
                mask[:, None, :].to_broadcast((P, 2, P)),
                mybir.AluOpType.mult)
        return pt, off

    def _normalize(l, j, pv):
        outT = outTs[j]
        # one recip covers both heads' denominators (partitions 0:63 of the
        # full 1024-wide pv tile) -- half the recip instructions
        rec = sm.tile([HD, 1024], F32, tag="rec", name="rec")
        nc.vector.reciprocal_approx_fast(rec[:], pv[0:HD, :])
        for hh in (0, 1):
            nc.vector.tensor_tensor(outT[hh * HD:(hh + 1) * HD, l, :],
                                    pv[HD:2 * HD, 512 * hh:512 * (hh + 1)],
                                    rec[:, 512 * hh:512 * (hh + 1)],
                                    mybir.AluOpType.mult)

    def proj_col_chain(j, m, pool=None):
        pool = pool if pool is not None else ps_pv
        ps = pool.tile([P, 1024], F32,
                       tag="sc" if pool is ps_sc else "pv", name="ps")[:, 0:512]
        for k in range(KP):
            nc.tensor.matmul(ps[:], wp[:, k, ts(m, P)], outTs[j][:, k, :],
                             start=(k == 0), stop=(k == KP - 1))
        yt = yout.tile([P, 512], BF16, tag="yt", name="yt")
        nc.vector.tensor_copy(yt[:], ps[:])
        # alternate DGE queues so the final column's stores drain in parallel
        eng = nc.sync if m % 2 == 0 else nc.gpsimd
        eng.dma_start(yT_t[:, m, ts(j, 512)], yt[:])

    def proj(j):
        # after the stream the score pool is idle: alternate pools so the
        # tail chains double their psum slots
        for m in range(8):
            proj_col_chain(j, m, pool=(ps_sc if m % 2 else ps_pv))

    class Pacer:
        # Bresenham-paced emission of filler matmul chains between
        # attention iterations, to keep the PE dense (HAM stays warm).
        # Urgent thunks (deferred softmax normalizes) fire one per tick
        # ahead of the paced stream so DVE recips interleave with, not
        # ahead of, the next pair's mask multiplies.
        def __init__(self, thunks, total_ticks):
            self.thunks = list(thunks)
            self.total = max(1, total_ticks)
            self.ticks = 0
            self.fired = 0
            self.urgent = []

        def inject(self, thunks):
            self.urgent.extend(thunks)

        def tick(self):
            self.ticks += 1
            if self.urgent:
                self.urgent.pop(0)()
                return
            while (self.fired < len(self.thunks)
                   and self.fired * self.total < self.ticks * len(self.thunks)):
                self.thunks[self.fired]()
                self.fired += 1

        def flush(self):
            for t in self.urgent:
                t()
            self.urgent = []
            while self.fired < len(self.thunks):
                self.thunks[self.fired]()
                self.fired += 1

    # prelude DMAs: wqk8 k-pairs + x8 k-pairs interleaved on the sync queue
    # (the qk chains' critical path); wv + bf16 x on the gpsimd DGE queue in
    # parallel; wp on the vector queue (needed last, at proj time).
    xt0 = xin.tile([P, KD, 512], BF16, tag="xt", name="xt")
    x8t0 = [x8in.tile([P, 2, 512], FP8, tag="x8t", name="x8t")
            for _ in range(KD // 2)]
    for kk in range(KD // 2):
        nc.sync.dma_start(wqk8s[kk][:], wqk8_t[:, 2 * kk:2 * kk + 2, :])
        nc.sync.dma_start(x8t0[kk][:], xT8_t[:, 2 * kk:2 * kk + 2, ts(0, 512)])
    for k in range(KD):
        nc.gpsimd.dma_start(wv[:, k, :], wv_t[:, k, :])
        nc.gpsimd.dma_start(xt0[:, k, :], xT_t[:, k, ts(0, 512)])
    xts[0] = xt0
    x8ts[0] = x8t0
    nc.scalar.dma_start(wp[:], wp_t)

    # prelude: QKV for the first s-block
    for m in range(8):
        qkv_qk_chain(0, m)
    for ss in range(4):
        qkv_v_chain(0, ss)

    # Flattened, software-pipelined attention stream across ALL columns.
    # Filler allocation: qkv(j+1) is pinned to column j (needed at column
    # j+1); the proj chains are all deferred to the LAST column, which has
    # 40% of the attention iterations (and hence PE slack) but no qkv work.
    for j in range(NJ):
        outTs[j] = ot_pool.tile([P, KP, 512], BF16, tag="outT", name="outT")
    pacers = []
    for j in range(NJ):
        thunks = []
        if j + 1 < NJ:
            for m in range(8):
                thunks.append(lambda n=j + 1, m=m: qkv_qk_chain(n, m))
            for ss in range(4):
                thunks.append(lambda n=j + 1, ss=ss: qkv_v_chain(n, ss))
        else:
            for jj in range(NJ - 1):
                for m in range(8):
                    thunks.append(lambda jj=jj, m=m: proj_col_chain(jj, m))
        pacers.append(Pacer(thunks, 4 * 4 * (j + 1)))

    all_items = [(j, l, i) for j in range(NJ) for l in range(4)
                 for i in range(4 * (j + 1))]
    load_x(1)
    pvs = {}

    def fire_pv(j, l, i, pt, off):
        imax = 4 * (j + 1)
        if i == 0:
            pvs[(j, l)] = ps_pv.tile([P, 1024], F32, tag="pv", name="pv")
        pv = pvs[(j, l)]
        nc.tensor.matmul(pv[:, off:512], vaug[:, i, 2 * l, :],
                         pt[:, off:512],
                         start=(i == 0), stop=(i == imax - 1))
        nc.tensor.matmul(pv[:, 512 + off:1024], vaug[:, i, 2 * l + 1, :],
                         pt[:, 512 + off:1024],
                         start=(i == 0), stop=(i == imax - 1))
        if i == imax - 1:
            _normalize(l, j, pvs.pop((j, l)))

    # PV runs LAG items behind score emission so it never consumes a pt that
    # the ACT/DVE side finished only nanoseconds earlier (full SBUF-access
    # latency exposure); the extra stage costs nothing but pt pool depth.
    LAG = 3
    pending = deque()
    j0, l0, i0 = all_items[0]
    pending.append((j0, l0, i0) + _emit_scores(l0, j0, i0))
    for idx, (j, l, i) in enumerate(all_items):
        if idx + 1 < len(all_items):
            jn, ln, i_n = all_items[idx + 1]
            if jn != j:
                # column boundary: the next column's scores read qk8 written
                # by this column's qkv chains -- flush them FIRST so the
                # in-order PE queue never waits on work queued behind it.
                pacers[j].flush()
                if jn + 1 < NJ:
                    load_x(jn + 1)
            pending.append((jn, ln, i_n) + _emit_scores(ln, jn, i_n))
        pacers[j].tick()
        while len(pending) > LAG:
            fire_pv(*pending.popleft())
    while pending:
        fire_pv(*pending.popleft())
    pacers[NJ - 1].flush()
    proj(NJ - 1)


_NC = None


def build_nc():
    global _NC
    if _NC is not None:
        return _NC
    nc = bacc.Bacc("TRN2", target_bir_lowering=False, debug=False)
    xT = nc.dram_tensor("xT", [D, S], BF16, kind="ExternalInput")
    xT8 = nc.dram_tensor("xT8", [D, S], FP8, kind="ExternalInput")
    wqk8T = nc.dram_tensor("wqk8T", [D, EQK], FP8, kind="ExternalInput")
    wvT = nc.dram_tensor("wvT", [D, DS], BF16, kind="ExternalInput")
    wprojT = nc.dram_tensor("wprojT", [DS, D], BF16, kind="ExternalInput")
    yT = nc.dram_tensor("yT", [D, S], BF16, kind="ExternalOutput")
    with tile.TileContext(nc) as tc:
        _emit(tc, xT.ap(), xT8.ap(), wqk8T.ap(), wvT.ap(), wprojT.ap(),
              yT.ap())
    nc.compile()
    _NC = nc
    return nc


def make_in_maps(x, w_attn, w_proj):
    x = np.ascontiguousarray(np.asarray(x, dtype=np.float32))
    w_attn = np.asarray(w_attn, dtype=np.float32)
    w_proj = np.asarray(w_proj, dtype=np.float32)
    in_maps = []
    for c in range(8):
        b, g = divmod(c, 2)
        rows = slice(g * DS, (g + 1) * DS)
        wq_c = w_attn[0 * D:1 * D][rows] * SW           # [512, 1024]
        wk_c = w_attn[1 * D:2 * D][rows] * SW
        wqk8_c = np.concatenate([wq_c, wk_c], axis=0)   # [1024, 1024]
        wv_c = w_attn[2 * D:3 * D][rows]                # [512, 1024]
        xTb = np.ascontiguousarray(x[b].T)
        in_maps.append({
            "xT": xTb.astype(ml_dtypes.bfloat16),
            "xT8": (xTb * SX).astype(ml_dtypes.float8_e4m3),
            "wqk8T": np.ascontiguousarray(wqk8_c.T).astype(
                ml_dtypes.float8_e4m3),
            "wvT": np.ascontiguousarray(wv_c.T).astype(ml_dtypes.bfloat16),
            "wprojT": np.ascontiguousarray(w_proj[:, rows].T).astype(
                ml_dtypes.bfloat16),  # [512, 1024]
        })
    return in_maps


def gather(results):
    y = np.empty((B, S, D), dtype=np.float32)
    for b in range(B):
        yT = (results[2 * b]["yT"].astype(np.float32)
              + results[2 * b + 1]["yT"].astype(np.float32))
        y[b] = yT.T
    return y


def run(x, w_attn, w_proj, trace=False, tmpdir=None):
    nc = build_nc()
    in_maps = make_in_maps(x, w_attn, w_proj)
    res = run_bass_kernel_spmd(nc, in_maps, list(range(8)),
                               trace=trace, tmpdir=tmpdir)
    return gather(res.results), res


def kernel(x, w_attn, w_proj):
    y, _ = run(x, w_attn, w_proj)
    return y
